# revision 49
# baseline (speedup 1.0000x reference)
"""Trainium2 Bass kernel for nn_BasicDeconvolutionBlock.

Reference computation:
    gathered = feats[in_map]                         # [K, M, Cin]
    contrib  = einsum('kmc,kcd->kmd', gathered, W)   # [K, M, Cout]
    out      = zeros([n_out, Cout]).at[out_map].add(contrib)
    y        = relu(batchnorm(out))                  # batch stats over n_out rows

Strategy (8 NeuronCores, SPMD):
  - Host routes each (k, m) pair to the core owning its output row
    (row blocks of n_out/8) and lowers the gather to im2col: a per-core
    channel-major slab slabT[64, slots] (bf16) holding feats rows in
    k-major slot order, streamed to SBUF with large contiguous DMAs.
  - GEMM: per 128-slot tile (single k per tile), matmul(lhsT=slab tile
    [64ch,128slots], rhs=W[k][64ch,64]) -> PSUM [128slots,64] f32;
    PSUM->SBUF bf16 drains alternate between DVE and Activation.
  - Scatter: gpsimd dma_scatter_add (CCE-add, int16 idx, bf16 payload
    at 256B row stride) accumulates contributions into one of NBANKS=3
    HBM banks, rotating PER SEGMENT so nearby calls have no WAW hazard
    and the desc-gen / DMA / sem-prop chain pipelines ~3 deep.
    Duplicate output rows inside one call race in hardware, so the host
    spaces a row's occurrences ~cnt/h apart inside each k group,
    swap-fixes residual in-segment duplicates, and halves the rare
    segments whose duplicates are pigeonhole-stuck.  Scatter calls are
    12*128 = 1536 indices; the SWDGE descriptor ring is enlarged via
    dynamic_dma_scratch_size=73728 (the stock 16 KiB ring is what
    capped calls at ~896 indices and wedged beyond it).
  - BN: fold the banks chunk-wise (strided 64-of-128-col reads),
    ones-matmul row sums + Square sums interleaved with the fold,
    [2,64] AllReduce across the 8 cores, normalize + ReLU into a bf16
    staging tile, write y (bf16); the host casts to f32.
"""

import os
import sys

import numpy as np

sys.path.insert(0, "/opt/trn_rl_repo")

import ml_dtypes  # noqa: E402

from concourse import bacc, bass, mybir  # noqa: E402
import concourse.tile as tile  # noqa: E402

BN_EPS = 1e-5
SEG_TILES = int(os.environ.get("DECONV_SEG_TILES", "12"))
SUPER_SEGS = int(os.environ.get("DECONV_SUPER_SEGS", "3"))
NBANKS = int(os.environ.get("DECONV_NBANKS", "3"))
# SWDGE descriptor-ring carveout (SBUF bytes; ring = bytes/16 descs).
# Must hold the in-flight scatter calls (NBANKS x SEG_TILES x 128 descs).
# The stock 16384 ring is why calls used to be capped at 896 indices.
DMA_SCRATCH = int(os.environ.get("DECONV_SCRATCH", "73728"))
F32 = mybir.dt.float32
BF16 = mybir.dt.bfloat16
I16 = mybir.dt.int16


def _roundup(x, m):
    return (x + m - 1) // m * m


def _order_group(rows):
    """Slot order for one (core, k) group: spread a row's occurrences
    ~cnt/h apart so same-call duplicates are rare. Returns a permutation
    of range(len(rows))."""
    n = len(rows)
    if n == 0:
        return np.empty(0, dtype=np.int64)
    order = np.argsort(rows, kind="stable")
    sr = rows[order]
    first = np.ones(n, dtype=bool)
    first[1:] = sr[1:] != sr[:-1]
    grp = np.cumsum(first) - 1                    # rank of unique row
    grp_start = np.maximum.accumulate(np.where(first, np.arange(n), 0))
    occ = np.arange(n) - grp_start                # occurrence index j
    # occurrence count h per element
    cnt_per_grp = np.bincount(grp)
    h = cnt_per_grp[grp]
    nuniq = cnt_per_grp.size
    key = (occ + grp / max(nuniq, 1)) / h
    final = np.argsort(key, kind="stable")
    return order[final]


def _fix_conflicts(svals, gvals, seg_bounds, group_bounds, dump_row):
    """Ensure no duplicate (non-dump) rows within any segment by swapping
    slots within their k-group. svals/gvals modified in place."""
    nslots = len(svals)
    nseg = len(seg_bounds) - 1
    seg_of = np.zeros(nslots, dtype=np.int64)
    for s in range(nseg):
        seg_of[seg_bounds[s]:seg_bounds[s + 1]] = s
    grp_of = np.zeros(nslots, dtype=np.int64)
    for g in range(len(group_bounds) - 1):
        grp_of[group_bounds[g]:group_bounds[g + 1]] = g

    # per-seg row sets
    seg_sets = [set() for _ in range(nseg)]
    conflicts = []
    failed = set()
    is_conflict = np.zeros(nslots, dtype=bool)
    for i in range(nslots):
        r = svals[i]
        if r == dump_row:
            continue
        ss = seg_sets[seg_of[i]]
        if r in ss:
            conflicts.append(i)
            is_conflict[i] = True
        else:
            ss.add(r)
    rng = np.random.default_rng(0)
    for i in conflicts:
        r = int(svals[i])
        g = grp_of[i]
        lo, hi = group_bounds[g], group_bounds[g + 1]
        placed = False
        cands = list(rng.integers(lo, hi, size=200)) + list(range(lo, hi))
        for j in cands:
            j = int(j)
            sj = seg_of[j]
            if sj == seg_of[i] or is_conflict[j]:
                continue
            rj = int(svals[j])
            # after swap: r goes to seg sj, rj comes to seg of i
            if r in seg_sets[sj]:
                continue
            if rj != dump_row:
                if rj == r or rj in seg_sets[seg_of[i]]:
                    continue
            # apply swap
            si = seg_of[i]
            if rj != dump_row:
                seg_sets[sj].discard(rj)
                seg_sets[si].add(rj)
            seg_sets[sj].add(r)
            svals[i], svals[j] = svals[j], svals[i]
            gvals[i], gvals[j] = gvals[j], gvals[i]
            is_conflict[i] = False
            placed = True
            break
        if not placed:
            failed.add(int(seg_of[i]))
    return len(conflicts), failed


def _route(in_map, out_map, n_out, n_cores):
    """Host-side routing. Returns plan + per-core slot arrays
    (gvals: feats row per slot or -1; svals: local out row per slot)."""
    K, M = in_map.shape
    rows_per_core = n_out // n_cores
    assert rows_per_core * n_cores == n_out
    acc_rows = _roundup(rows_per_core, 128)
    dump_row = acc_rows
    acc_total = acc_rows + 128

    in_flat = in_map.ravel().astype(np.int64)
    out_flat = out_map.ravel().astype(np.int64)
    k_idx = np.repeat(np.arange(K, dtype=np.int64), M)
    core = out_flat // rows_per_core
    row_local = (out_flat - core * rows_per_core).astype(np.int64)

    # per (core, k) counts -> shared caps
    counts = np.zeros((n_cores, K), dtype=np.int64)
    np.add.at(counts, (core, k_idx), 1)
    caps = _roundup(counts.max(axis=0), 128)  # [K]
    group_bounds = np.concatenate([[0], np.cumsum(caps)])
    total_slots = int(group_bounds[-1])

    seg_slots = SEG_TILES * 128
    seg_bounds = list(range(0, total_slots, seg_slots)) + [total_slots]
    if seg_bounds[-1] == seg_bounds[-2]:
        seg_bounds.pop()

    # iteratively split segments whose duplicate conflicts can't be
    # swap-fixed (rare: rows occupying every window their group spans)
    for _ in range(8):
        per_core = []
        all_failed = set()
        for c in range(n_cores):
            gvals = np.full(total_slots, -1, dtype=np.int64)
            svals = np.full(total_slots, dump_row, dtype=np.int64)
            sel_c = core == c
            for k in range(K):
                sel = np.nonzero(sel_c & (k_idx == k))[0]
                rows_k = row_local[sel]
                perm = _order_group(rows_k)
                g0 = group_bounds[k]
                n = len(sel)
                gvals[g0:g0 + n] = in_flat[sel][perm]
                svals[g0:g0 + n] = rows_k[perm]
            nfix, failed = _fix_conflicts(svals, gvals, seg_bounds,
                                          group_bounds, dump_row)
            per_core.append((gvals, svals, nfix))
            all_failed |= failed
        if not all_failed:
            break
        new_bounds = []
        for s in range(len(seg_bounds) - 1):
            a, b = seg_bounds[s], seg_bounds[s + 1]
            new_bounds.append(a)
            if s in all_failed and b - a >= 256:
                new_bounds.append(a + (b - a) // 256 * 128)
        new_bounds.append(seg_bounds[-1])
        seg_bounds = new_bounds
    else:
        raise RuntimeError("segment splitting did not converge")

    # tile -> k map
    ntiles = total_slots // 128
    tile_k = np.zeros(ntiles, dtype=np.int64)
    for k in range(K):
        tile_k[group_bounds[k] // 128:group_bounds[k + 1] // 128] = k

    plan = dict(
        K=K, rows_per_core=rows_per_core, acc_rows=acc_rows,
        acc_total=acc_total, dump_row=dump_row,
        total_slots=total_slots, ntiles=ntiles, tile_k=tile_k,
        seg_bounds=seg_bounds, seg_slots=seg_slots,
    )
    return plan, per_core


def _build(plan, n_out, n_cores):
    """Trace the Bass program. Returns nc."""
    nc = bacc.Bacc("TRN2", target_bir_lowering=False, debug=False,
                   dynamic_dma_scratch_size=DMA_SCRATCH)

    K = plan["K"]
    acc_rows, acc_total = plan["acc_rows"], plan["acc_total"]
    total_slots = plan["total_slots"]
    tile_k = plan["tile_k"]
    seg_bounds = plan["seg_bounds"]
    nseg = len(seg_bounds) - 1
    Cout = 64

    slabt = nc.dram_tensor("slabt", [64, total_slots], BF16,
                           kind="ExternalInput")
    wt = nc.dram_tensor("wt", [64, K * Cout], BF16, kind="ExternalInput")
    sidx = nc.dram_tensor("sidx", [128, total_slots // 16], I16,
                          kind="ExternalInput")
    gb = nc.dram_tensor("gb", [2, Cout], F32, kind="ExternalInput")
    # bf16 accumulator banks, rows padded to 128 cols so the scatter's
    # 256B row stride holds (elem_step=128, payload=64 cols)
    accs = [nc.dram_tensor(f"acc{b}", [acc_total, 128], BF16)
            for b in range(NBANKS)]
    cc_in = nc.dram_tensor("cc_in", [2, Cout], F32)
    cc_out = nc.dram_tensor("cc_out", [2, Cout], F32, addr_space="Shared")
    y = nc.dram_tensor("y", [acc_rows, Cout], BF16, kind="ExternalOutput")

    Tb = acc_rows // 128  # BN column tiles

    # super-segment layout: SUPER_SEGS segments per slab load
    supers = []
    s = 0
    while s < nseg:
        e = min(s + SUPER_SEGS, nseg)
        supers.append((s, e))
        s = e

    with tile.TileContext(nc) as tc:
        with (
            tc.tile_pool(name="const", bufs=1) as cpool,
            tc.tile_pool(name="slab", bufs=4) as slabpool,
            tc.tile_pool(name="oslab", bufs=8) as opool,
            tc.tile_pool(name="sixp", bufs=4) as sixpool,
            tc.tile_pool(name="psum", bufs=8, space="PSUM") as pspool,
        ):
            w_sb = cpool.tile([64, K * Cout], BF16, tag="w")
            nc.sync.dma_start(out=w_sb[:, :], in_=wt[:, :])
            zed = cpool.tile([128, 3200], BF16, tag="zed")
            nc.vector.memset(zed[:, :], 0.0)
            zrows = 6400  # rows per DMA (only the used 64 cols are zeroed)
            # zero acc0 first (gates the first scatter); later banks'
            # zeros are emitted after so they overlap the first supers
            for r0 in range(0, acc_total, zrows):
                rcnt = min(zrows, acc_total - r0)
                nc.sync.dma_start(
                    out=accs[0][r0:r0 + rcnt, 0:64],
                    in_=zed[:, :rcnt // 2],
                )

            zero_chunks = [(bank, r0) for bank in accs[1:]
                           for r0 in range(0, acc_total, zrows)]
            for (s0seg, s1seg) in supers:
                a = seg_bounds[s0seg]
                b = seg_bounds[s1seg]
                ns_sup = b - a
                g = slabpool.tile([64, SUPER_SEGS * plan["seg_slots"]],
                                  BF16, tag="g")
                nc.sync.dma_start(out=g[:, :ns_sup], in_=slabt[:, a:b])
                si_t = sixpool.tile(
                    [128, SUPER_SEGS * plan["seg_slots"] // 16], I16,
                    tag="si")
                nc.sync.dma_start(
                    out=si_t[:, :ns_sup // 16],
                    in_=sidx[:, a // 16:b // 16],
                )
                while zero_chunks:
                    # remaining banks' zeros overlap the first super
                    bank, r0 = zero_chunks.pop(0)
                    rcnt = min(zrows, acc_total - r0)
                    nc.sync.dma_start(
                        out=bank[r0:r0 + rcnt, 0:64],
                        in_=zed[:, :rcnt // 2],
                    )
                for seg in range(s0seg, s1seg):
                    sa = seg_bounds[seg]
                    sb = seg_bounds[seg + 1]
                    ns = sb - sa
                    ntile = ns // 128
                    oslab = opool.tile([128, SEG_TILES, Cout], BF16,
                                       tag="oslab")
                    for t in range(ntile):
                        col = (sa - a) + t * 128
                        k = int(tile_k[sa // 128 + t])
                        ps = pspool.tile([128, Cout], F32, tag="ps")
                        nc.tensor.matmul(
                            out=ps[:, :],
                            lhsT=g[:, col:col + 128],
                            rhs=w_sb[:, k * Cout:(k + 1) * Cout],
                            start=True, stop=True,
                        )
                        if t % 2 == 0:  # split PSUM drains across DVE/Act
                            nc.vector.tensor_copy(
                                out=oslab[:, t, :], in_=ps[:, :])
                        else:
                            nc.scalar.activation(
                                out=oslab[:, t, :], in_=ps[:, :],
                                func=mybir.ActivationFunctionType.Copy)
                    nc.gpsimd.dma_scatter_add(
                        out_ap=accs[seg % NBANKS][:, 0:Cout],
                        in_ap=oslab[:, :ntile, :],
                        idxs_ap=si_t[:, (sa - a) // 16:(sb - a) // 16],
                        num_idxs=ns,
                        num_idxs_reg=ns,
                        elem_size=Cout,
                        elem_step=128,
                    )

        # ---- BN phase ----
        with (
            tc.tile_pool(name="bn", bufs=1) as bnpool,
            tc.tile_pool(name="bns", bufs=4) as bnspool,
            tc.tile_pool(name="bnp", bufs=2, space="PSUM") as bnps,
        ):
            out_sb = bnpool.tile([128, Tb, 64], BF16, tag="outsb")
            ones = bnpool.tile([128, 1], BF16, tag="ones")
            nc.vector.memset(ones[:, :], 1.0)
            sum_ps = bnps.tile([1, 64], F32, tag="sum")
            sq_ps = bnps.tile([1, 64], F32, tag="sq")
            CH = 49  # fold chunk (tiles of 128 rows)
            with (
                tc.tile_pool(name="bnc", bufs=2 * NBANKS) as bncpool,
                tc.tile_pool(name="bnsq", bufs=4) as sqpool,
            ):
                for c0 in range(0, Tb, CH):
                    cc = min(CH, Tb - c0)
                    chunk_sbs = []
                    for b in range(NBANKS):
                        bsb = bncpool.tile([128, CH, 64], BF16, tag="bchunk")
                        nc.sync.dma_start(
                            out=bsb[:, :cc, :],
                            in_=accs[b][c0 * 128:(c0 + cc) * 128, 0:64])
                        chunk_sbs.append(bsb)
                    nc.vector.tensor_tensor(
                        out=out_sb[:, c0:c0 + cc, :],
                        in0=chunk_sbs[0][:, :cc, :],
                        in1=chunk_sbs[1][:, :cc, :], op=mybir.AluOpType.add)
                    for b in range(2, NBANKS):
                        nc.vector.tensor_tensor(
                            out=out_sb[:, c0:c0 + cc, :],
                            in0=out_sb[:, c0:c0 + cc, :],
                            in1=chunk_sbs[b][:, :cc, :],
                            op=mybir.AluOpType.add)
                    # interleave the stats reductions with the fold
                    for t in range(c0, c0 + cc):
                        sqt = sqpool.tile([128, 64], BF16, tag="sqt")
                        if t % 4 == 0:
                            nc.vector.tensor_tensor(
                                out=sqt[:, :], in0=out_sb[:, t, :],
                                in1=out_sb[:, t, :], op=mybir.AluOpType.mult)
                        else:
                            nc.scalar.activation(
                                out=sqt[:, :], in_=out_sb[:, t, :],
                                func=mybir.ActivationFunctionType.Square)
                        nc.tensor.matmul(
                            out=sum_ps[:, :], lhsT=ones[:, :],
                            rhs=out_sb[:, t, :],
                            start=(t == 0), stop=(t == Tb - 1),
                        )
                        nc.tensor.matmul(
                            out=sq_ps[:, :], lhsT=ones[:, :], rhs=sqt[:, :],
                            start=(t == 0), stop=(t == Tb - 1),
                        )
            st0 = bnspool.tile([1, 64], F32, tag="st0")
            st1 = bnspool.tile([1, 64], F32, tag="st1")
            nc.vector.tensor_copy(out=st0[:, :], in_=sum_ps[:, :])
            nc.vector.tensor_copy(out=st1[:, :], in_=sq_ps[:, :])
            nc.sync.dma_start(out=cc_in[0:1, :], in_=st0[:, :])
            nc.sync.dma_start(out=cc_in[1:2, :], in_=st1[:, :])
            nc.gpsimd.collective_compute(
                "AllReduce",
                mybir.AluOpType.add,
                ins=[cc_in[:, :]],
                outs=[cc_out[:, :]],
                replica_groups=[list(range(n_cores))],
            )
            gs0 = bnspool.tile([1, 64], F32, tag="gs0")
            gs1 = bnspool.tile([1, 64], F32, tag="gs1")
            nc.sync.dma_start(out=gs0[:, :], in_=cc_out[0:1, :])
            nc.sync.dma_start(out=gs1[:, :], in_=cc_out[1:2, :])
            gam_t = bnspool.tile([1, 64], F32, tag="gam")
            bet_t = bnspool.tile([1, 64], F32, tag="bet")
            nc.sync.dma_start(out=gam_t[:, :], in_=gb[0:1, :])
            nc.sync.dma_start(out=bet_t[:, :], in_=gb[1:2, :])

            inv_n = 1.0 / float(n_out)
            mean_t = bnspool.tile([1, 64], F32, tag="mean")
            ex2_t = bnspool.tile([1, 64], F32, tag="ex2")
            var_t = bnspool.tile([1, 64], F32, tag="var")
            sd_t = bnspool.tile([1, 64], F32, tag="sd")
            rs_t = bnspool.tile([1, 64], F32, tag="rs")
            a_t = bnspool.tile([1, 64], F32, tag="a")
            b_t = bnspool.tile([1, 64], F32, tag="b")
            nc.vector.tensor_scalar_mul(mean_t[:, :], gs0[:, :], inv_n)
            nc.vector.tensor_scalar_mul(ex2_t[:, :], gs1[:, :], inv_n)
            nc.vector.tensor_tensor(
                out=var_t[:, :], in0=mean_t[:, :], in1=mean_t[:, :],
                op=mybir.AluOpType.mult)
            nc.vector.tensor_tensor(
                out=var_t[:, :], in0=ex2_t[:, :], in1=var_t[:, :],
                op=mybir.AluOpType.subtract)
            nc.vector.tensor_scalar_add(var_t[:, :], var_t[:, :], BN_EPS)
            nc.scalar.activation(
                out=sd_t[:, :], in_=var_t[:, :],
                func=mybir.ActivationFunctionType.Sqrt)
            nc.vector.reciprocal(out=rs_t[:, :], in_=sd_t[:, :])
            nc.vector.tensor_tensor(
                out=a_t[:, :], in0=gam_t[:, :], in1=rs_t[:, :],
                op=mybir.AluOpType.mult)
            nc.vector.tensor_tensor(
                out=b_t[:, :], in0=mean_t[:, :], in1=a_t[:, :],
                op=mybir.AluOpType.mult)
            nc.vector.tensor_tensor(
                out=b_t[:, :], in0=bet_t[:, :], in1=b_t[:, :],
                op=mybir.AluOpType.subtract)
            # broadcast [1,64] -> [128,64] via PE (ones[1,128]^T @ row)
            ones_row = bnspool.tile([1, 128], F32, tag="ones_row")
            nc.vector.memset(ones_row[:, :], 1.0)
            a_full = bnspool.tile([128, 64], BF16, tag="afull")
            b_full = bnspool.tile([128, 64], BF16, tag="bfull")
            ab_ps = bnps.tile([128, 64], F32, tag="abps")
            nc.tensor.matmul(
                out=ab_ps[:, :], lhsT=ones_row[:, :], rhs=a_t[:, :],
                start=True, stop=True)
            nc.vector.tensor_copy(out=a_full[:, :], in_=ab_ps[:, :])
            nc.tensor.matmul(
                out=ab_ps[:, :], lhsT=ones_row[:, :], rhs=b_t[:, :],
                start=True, stop=True)
            nc.vector.tensor_copy(out=b_full[:, :], in_=ab_ps[:, :])
            # normalize in bf16, relu converts to f32 staging, write chunks
            with tc.tile_pool(name="bny", bufs=2) as ypool:
                for c0 in range(0, Tb, CH):
                    cc = min(CH, Tb - c0)
                    stage = ypool.tile([128, CH, 64], BF16, tag="stage")
                    for t in range(c0, c0 + cc):
                        nc.vector.tensor_tensor(
                            out=out_sb[:, t, :], in0=out_sb[:, t, :],
                            in1=a_full[:, :], op=mybir.AluOpType.mult)
                        nc.vector.tensor_tensor(
                            out=out_sb[:, t, :], in0=out_sb[:, t, :],
                            in1=b_full[:, :], op=mybir.AluOpType.add)
                        if t % 3 == 0:  # relu split: keep Act the short pole
                            nc.vector.tensor_scalar_max(
                                stage[:, t - c0, :], out_sb[:, t, :], 0.0)
                        else:
                            nc.scalar.activation(
                                out=stage[:, t - c0, :], in_=out_sb[:, t, :],
                                func=mybir.ActivationFunctionType.Relu)
                    nc.sync.dma_start(
                        out=y[c0 * 128:(c0 + cc) * 128, :],
                        in_=stage[:, :cc, :])

    nc.compile()
    return nc


def _pack_sidx(svals):
    """[total_slots] int -> [128, total_slots//16] int16 wrapped/tiled."""
    cols = svals.reshape(-1, 16).T.astype(np.int16)  # [16, n/16]
    return np.tile(cols, (8, 1))


def _prepare(feats, W, gamma, beta, in_map, out_map, n_out, n_cores=8,
             *_ignored):
    """Host prep shared by kernel() and tests. Returns (nc, in_maps, plan)."""
    n_out = int(n_out)
    K, Cin, Cout = W.shape
    assert Cin == 64 and Cout == 64
    in_map = np.asarray(in_map, dtype=np.int64)
    out_map = np.asarray(out_map, dtype=np.int64)
    feats = np.asarray(feats, dtype=np.float32)
    W = np.asarray(W, dtype=np.float32)

    plan, per_core = _route(in_map, out_map, n_out, n_cores)

    featsT = np.ascontiguousarray(
        feats.T.astype(ml_dtypes.bfloat16))          # [64, N_in]
    featsT_pad = np.concatenate(
        [featsT, np.zeros((64, 1), dtype=ml_dtypes.bfloat16)], axis=1)

    wt = np.ascontiguousarray(
        W.transpose(1, 0, 2).reshape(64, K * 64).astype(ml_dtypes.bfloat16))

    gb = np.stack([np.asarray(gamma, np.float32),
                   np.asarray(beta, np.float32)])

    nc = _build(plan, n_out, n_cores)
    in_maps = []
    for c in range(n_cores):
        gvals, svals, _ = per_core[c]
        slabt = featsT_pad[:, gvals]                 # -1 -> zero column
        in_maps.append(dict(slabt=np.ascontiguousarray(slabt), wt=wt,
                            sidx=_pack_sidx(svals), gb=gb))
    return nc, in_maps, plan


def kernel(feats, W, gamma, beta, in_map, out_map, n_out):
    from concourse.bass_utils import run_bass_kernel_spmd

    n_cores = 8
    nc, in_maps, plan = _prepare(
        feats, W, gamma, beta, in_map, out_map, n_out, n_cores)
    res = run_bass_kernel_spmd(nc, in_maps, list(range(n_cores)))
    rows = plan["rows_per_core"]
    out = np.concatenate(
        [res.results[c]["y"][:rows] for c in range(n_cores)], axis=0)
    return out.astype(np.float32)


# revision 50
# speedup vs baseline: 1.0812x; 1.0812x over previous
"""Trainium2 Bass kernel for nn_BasicDeconvolutionBlock.

Reference computation:
    gathered = feats[in_map]                         # [K, M, Cin]
    contrib  = einsum('kmc,kcd->kmd', gathered, W)   # [K, M, Cout]
    out      = zeros([n_out, Cout]).at[out_map].add(contrib)
    y        = relu(batchnorm(out))                  # batch stats over n_out rows

Strategy (8 NeuronCores, SPMD):
  - Host routes each (k, m) pair to the core owning its output row
    (row blocks of n_out/8) and lowers the gather to im2col: a per-core
    channel-major slab slabT[64, slots] (bf16) holding feats rows in
    k-major slot order, streamed to SBUF with large contiguous DMAs.
  - GEMM: per 128-slot tile (single k per tile), matmul(lhsT=slab tile
    [64ch,128slots], rhs=W[k][64ch,64]) -> PSUM [128slots,64] f32;
    PSUM->SBUF bf16 drains alternate between DVE and Activation.
  - Scatter: gpsimd dma_scatter_add (CCE-add, int16 idx, bf16 payload
    at 256B row stride) accumulates contributions into one of NBANKS=3
    HBM banks, rotating PER SEGMENT so nearby calls have no WAW hazard
    and the desc-gen / DMA / sem-prop chain pipelines ~3 deep.
    Duplicate output rows inside one call race in hardware, so the host
    spaces a row's occurrences ~cnt/h apart inside each k group,
    swap-fixes residual in-segment duplicates, and halves the rare
    segments whose duplicates are pigeonhole-stuck.  Scatter calls are
    12*128 = 1536 indices; the SWDGE descriptor ring is enlarged via
    dynamic_dma_scratch_size=73728 (the stock 16 KiB ring is what
    capped calls at ~896 indices and wedged beyond it).
  - BN: fold the banks chunk-wise (strided 64-of-128-col reads),
    ones-matmul row sums + Square sums interleaved with the fold,
    [2,64] AllReduce across the 8 cores, normalize + ReLU into a bf16
    staging tile, write y (bf16); the host casts to f32.
"""

import os
import sys

import numpy as np

sys.path.insert(0, "/opt/trn_rl_repo")

import ml_dtypes  # noqa: E402

from concourse import bacc, bass, mybir  # noqa: E402
import concourse.tile as tile  # noqa: E402

BN_EPS = 1e-5
SEG_TILES = int(os.environ.get("DECONV_SEG_TILES", "12"))
SUPER_SEGS = int(os.environ.get("DECONV_SUPER_SEGS", "2"))
NBANKS = int(os.environ.get("DECONV_NBANKS", "3"))
# SWDGE descriptor-ring carveout (SBUF bytes; ring = bytes/16 descs).
# Must hold the in-flight scatter calls (NBANKS x SEG_TILES x 128 descs).
# The stock 16384 ring is why calls used to be capped at 896 indices.
DMA_SCRATCH = int(os.environ.get("DECONV_SCRATCH", "73728"))
F32 = mybir.dt.float32
BF16 = mybir.dt.bfloat16
I16 = mybir.dt.int16


def _roundup(x, m):
    return (x + m - 1) // m * m


def _order_group(rows):
    """Slot order for one (core, k) group: spread a row's occurrences
    ~cnt/h apart so same-call duplicates are rare. Returns a permutation
    of range(len(rows))."""
    n = len(rows)
    if n == 0:
        return np.empty(0, dtype=np.int64)
    order = np.argsort(rows, kind="stable")
    sr = rows[order]
    first = np.ones(n, dtype=bool)
    first[1:] = sr[1:] != sr[:-1]
    grp = np.cumsum(first) - 1                    # rank of unique row
    grp_start = np.maximum.accumulate(np.where(first, np.arange(n), 0))
    occ = np.arange(n) - grp_start                # occurrence index j
    # occurrence count h per element
    cnt_per_grp = np.bincount(grp)
    h = cnt_per_grp[grp]
    nuniq = cnt_per_grp.size
    key = (occ + grp / max(nuniq, 1)) / h
    final = np.argsort(key, kind="stable")
    return order[final]


def _fix_conflicts(svals, gvals, seg_bounds, group_bounds, dump_row):
    """Ensure no duplicate (non-dump) rows within any segment by swapping
    slots within their k-group. svals/gvals modified in place."""
    nslots = len(svals)
    nseg = len(seg_bounds) - 1
    seg_of = np.zeros(nslots, dtype=np.int64)
    for s in range(nseg):
        seg_of[seg_bounds[s]:seg_bounds[s + 1]] = s
    grp_of = np.zeros(nslots, dtype=np.int64)
    for g in range(len(group_bounds) - 1):
        grp_of[group_bounds[g]:group_bounds[g + 1]] = g

    # per-seg row sets
    seg_sets = [set() for _ in range(nseg)]
    conflicts = []
    failed = set()
    is_conflict = np.zeros(nslots, dtype=bool)
    for i in range(nslots):
        r = svals[i]
        if r == dump_row:
            continue
        ss = seg_sets[seg_of[i]]
        if r in ss:
            conflicts.append(i)
            is_conflict[i] = True
        else:
            ss.add(r)
    rng = np.random.default_rng(0)
    for i in conflicts:
        r = int(svals[i])
        g = grp_of[i]
        lo, hi = group_bounds[g], group_bounds[g + 1]
        placed = False
        cands = list(rng.integers(lo, hi, size=200)) + list(range(lo, hi))
        for j in cands:
            j = int(j)
            sj = seg_of[j]
            if sj == seg_of[i] or is_conflict[j]:
                continue
            rj = int(svals[j])
            # after swap: r goes to seg sj, rj comes to seg of i
            if r in seg_sets[sj]:
                continue
            if rj != dump_row:
                if rj == r or rj in seg_sets[seg_of[i]]:
                    continue
            # apply swap
            si = seg_of[i]
            if rj != dump_row:
                seg_sets[sj].discard(rj)
                seg_sets[si].add(rj)
            seg_sets[sj].add(r)
            svals[i], svals[j] = svals[j], svals[i]
            gvals[i], gvals[j] = gvals[j], gvals[i]
            is_conflict[i] = False
            placed = True
            break
        if not placed:
            failed.add(int(seg_of[i]))
    return len(conflicts), failed


def _route(in_map, out_map, n_out, n_cores):
    """Host-side routing. Returns plan + per-core slot arrays
    (gvals: feats row per slot or -1; svals: local out row per slot)."""
    K, M = in_map.shape
    rows_per_core = n_out // n_cores
    assert rows_per_core * n_cores == n_out
    acc_rows = _roundup(rows_per_core, 128)
    dump_row = acc_rows
    acc_total = acc_rows + 128

    in_flat = in_map.ravel().astype(np.int64)
    out_flat = out_map.ravel().astype(np.int64)
    k_idx = np.repeat(np.arange(K, dtype=np.int64), M)
    core = out_flat // rows_per_core
    row_local = (out_flat - core * rows_per_core).astype(np.int64)

    # per (core, k) counts -> shared caps
    counts = np.zeros((n_cores, K), dtype=np.int64)
    np.add.at(counts, (core, k_idx), 1)
    caps = _roundup(counts.max(axis=0), 128)  # [K]
    group_bounds = np.concatenate([[0], np.cumsum(caps)])
    total_slots = int(group_bounds[-1])

    seg_slots = SEG_TILES * 128
    seg_bounds = list(range(0, total_slots, seg_slots)) + [total_slots]
    if seg_bounds[-1] == seg_bounds[-2]:
        seg_bounds.pop()

    # iteratively split segments whose duplicate conflicts can't be
    # swap-fixed (rare: rows occupying every window their group spans)
    for _ in range(8):
        per_core = []
        all_failed = set()
        for c in range(n_cores):
            gvals = np.full(total_slots, -1, dtype=np.int64)
            svals = np.full(total_slots, dump_row, dtype=np.int64)
            sel_c = core == c
            for k in range(K):
                sel = np.nonzero(sel_c & (k_idx == k))[0]
                rows_k = row_local[sel]
                perm = _order_group(rows_k)
                g0 = group_bounds[k]
                n = len(sel)
                gvals[g0:g0 + n] = in_flat[sel][perm]
                svals[g0:g0 + n] = rows_k[perm]
            nfix, failed = _fix_conflicts(svals, gvals, seg_bounds,
                                          group_bounds, dump_row)
            per_core.append((gvals, svals, nfix))
            all_failed |= failed
        if not all_failed:
            break
        new_bounds = []
        for s in range(len(seg_bounds) - 1):
            a, b = seg_bounds[s], seg_bounds[s + 1]
            new_bounds.append(a)
            if s in all_failed and b - a >= 256:
                new_bounds.append(a + (b - a) // 256 * 128)
        new_bounds.append(seg_bounds[-1])
        seg_bounds = new_bounds
    else:
        raise RuntimeError("segment splitting did not converge")

    # tile -> k map
    ntiles = total_slots // 128
    tile_k = np.zeros(ntiles, dtype=np.int64)
    for k in range(K):
        tile_k[group_bounds[k] // 128:group_bounds[k + 1] // 128] = k

    plan = dict(
        K=K, rows_per_core=rows_per_core, acc_rows=acc_rows,
        acc_total=acc_total, dump_row=dump_row,
        total_slots=total_slots, ntiles=ntiles, tile_k=tile_k,
        seg_bounds=seg_bounds, seg_slots=seg_slots,
    )
    return plan, per_core


def _build(plan, n_out, n_cores):
    """Trace the Bass program. Returns nc."""
    nc = bacc.Bacc("TRN2", target_bir_lowering=False, debug=False,
                   dynamic_dma_scratch_size=DMA_SCRATCH)

    K = plan["K"]
    acc_rows, acc_total = plan["acc_rows"], plan["acc_total"]
    total_slots = plan["total_slots"]
    tile_k = plan["tile_k"]
    seg_bounds = plan["seg_bounds"]
    nseg = len(seg_bounds) - 1
    Cout = 64

    slabt = nc.dram_tensor("slabt", [64, total_slots], BF16,
                           kind="ExternalInput")
    wt = nc.dram_tensor("wt", [64, K * Cout], BF16, kind="ExternalInput")
    sidx = nc.dram_tensor("sidx", [128, total_slots // 16], I16,
                          kind="ExternalInput")
    gb = nc.dram_tensor("gb", [2, Cout], F32, kind="ExternalInput")
    # bf16 accumulator banks, rows padded to 128 cols so the scatter's
    # 256B row stride holds (elem_step=128, payload=64 cols)
    accs = [nc.dram_tensor(f"acc{b}", [acc_total, 128], BF16)
            for b in range(NBANKS)]
    cc_in = nc.dram_tensor("cc_in", [2, Cout], F32)
    cc_out = nc.dram_tensor("cc_out", [2, Cout], F32, addr_space="Shared")
    y = nc.dram_tensor("y", [acc_rows, Cout], BF16, kind="ExternalOutput")

    Tb = acc_rows // 128  # BN column tiles

    # super-segment layout: SUPER_SEGS segments per slab load
    supers = []
    s = 0
    while s < nseg:
        e = min(s + SUPER_SEGS, nseg)
        supers.append((s, e))
        s = e

    with tile.TileContext(nc) as tc:
        with (
            tc.tile_pool(name="const", bufs=1) as cpool,
            tc.tile_pool(name="slab", bufs=4) as slabpool,
            tc.tile_pool(name="oslab", bufs=8) as opool,
            tc.tile_pool(name="sixp", bufs=4) as sixpool,
            tc.tile_pool(name="psum", bufs=8, space="PSUM") as pspool,
        ):
            w_sb = cpool.tile([64, K * Cout], BF16, tag="w")
            nc.sync.dma_start(out=w_sb[:, :], in_=wt[:, :])
            zed = cpool.tile([128, 3200], BF16, tag="zed")
            nc.vector.memset(zed[:, :], 0.0)
            zrows = 6400  # rows per DMA (only the used 64 cols are zeroed)
            # zero acc0 first (gates the first scatter); later banks'
            # zeros are emitted after so they overlap the first supers
            for r0 in range(0, acc_total, zrows):
                rcnt = min(zrows, acc_total - r0)
                nc.sync.dma_start(
                    out=accs[0][r0:r0 + rcnt, 0:64],
                    in_=zed[:, :rcnt // 2],
                )

            zero_chunks = [(bank, r0) for bank in accs[1:]
                           for r0 in range(0, acc_total, zrows)]
            for (s0seg, s1seg) in supers:
                a = seg_bounds[s0seg]
                b = seg_bounds[s1seg]
                ns_sup = b - a
                g = slabpool.tile([64, SUPER_SEGS * plan["seg_slots"]],
                                  BF16, tag="g")
                nc.sync.dma_start(out=g[:, :ns_sup], in_=slabt[:, a:b])
                si_t = sixpool.tile(
                    [128, SUPER_SEGS * plan["seg_slots"] // 16], I16,
                    tag="si")
                nc.sync.dma_start(
                    out=si_t[:, :ns_sup // 16],
                    in_=sidx[:, a // 16:b // 16],
                )
                while zero_chunks:
                    # remaining banks' zeros overlap the first super
                    bank, r0 = zero_chunks.pop(0)
                    rcnt = min(zrows, acc_total - r0)
                    nc.sync.dma_start(
                        out=bank[r0:r0 + rcnt, 0:64],
                        in_=zed[:, :rcnt // 2],
                    )
                for seg in range(s0seg, s1seg):
                    sa = seg_bounds[seg]
                    sb = seg_bounds[seg + 1]
                    ns = sb - sa
                    ntile = ns // 128
                    oslab = opool.tile([128, SEG_TILES, Cout], BF16,
                                       tag="oslab")
                    for t in range(ntile):
                        col = (sa - a) + t * 128
                        k = int(tile_k[sa // 128 + t])
                        ps = pspool.tile([128, Cout], F32, tag="ps")
                        nc.tensor.matmul(
                            out=ps[:, :],
                            lhsT=g[:, col:col + 128],
                            rhs=w_sb[:, k * Cout:(k + 1) * Cout],
                            start=True, stop=True,
                        )
                        if t % 2 == 0:  # split PSUM drains across DVE/Act
                            nc.vector.tensor_copy(
                                out=oslab[:, t, :], in_=ps[:, :])
                        else:
                            nc.scalar.activation(
                                out=oslab[:, t, :], in_=ps[:, :],
                                func=mybir.ActivationFunctionType.Copy)
                    nc.gpsimd.dma_scatter_add(
                        out_ap=accs[seg % NBANKS][:, 0:Cout],
                        in_ap=oslab[:, :ntile, :],
                        idxs_ap=si_t[:, (sa - a) // 16:(sb - a) // 16],
                        num_idxs=ns,
                        num_idxs_reg=ns,
                        elem_size=Cout,
                        elem_step=128,
                    )

        # ---- BN phase ----
        with (
            tc.tile_pool(name="bn", bufs=1) as bnpool,
            tc.tile_pool(name="bns", bufs=4) as bnspool,
            tc.tile_pool(name="bnp", bufs=2, space="PSUM") as bnps,
        ):
            out_sb = bnpool.tile([128, Tb, 64], BF16, tag="outsb")
            ones = bnpool.tile([128, 1], BF16, tag="ones")
            nc.vector.memset(ones[:, :], 1.0)
            sum_ps = bnps.tile([1, 64], F32, tag="sum")
            sq_ps = bnps.tile([1, 64], F32, tag="sq")
            CH = 49  # fold chunk (tiles of 128 rows)
            with (
                tc.tile_pool(name="bnc", bufs=2 * NBANKS) as bncpool,
                tc.tile_pool(name="bnsq", bufs=4) as sqpool,
            ):
                for c0 in range(0, Tb, CH):
                    cc = min(CH, Tb - c0)
                    chunk_sbs = []
                    for b in range(NBANKS):
                        bsb = bncpool.tile([128, CH, 64], BF16, tag="bchunk")
                        nc.sync.dma_start(
                            out=bsb[:, :cc, :],
                            in_=accs[b][c0 * 128:(c0 + cc) * 128, 0:64])
                        chunk_sbs.append(bsb)
                    nc.vector.tensor_tensor(
                        out=out_sb[:, c0:c0 + cc, :],
                        in0=chunk_sbs[0][:, :cc, :],
                        in1=chunk_sbs[1][:, :cc, :], op=mybir.AluOpType.add)
                    for b in range(2, NBANKS):
                        nc.vector.tensor_tensor(
                            out=out_sb[:, c0:c0 + cc, :],
                            in0=out_sb[:, c0:c0 + cc, :],
                            in1=chunk_sbs[b][:, :cc, :],
                            op=mybir.AluOpType.add)
                    # interleave the stats reductions with the fold
                    for t in range(c0, c0 + cc):
                        sqt = sqpool.tile([128, 64], BF16, tag="sqt")
                        if t % 4 == 0:
                            nc.vector.tensor_tensor(
                                out=sqt[:, :], in0=out_sb[:, t, :],
                                in1=out_sb[:, t, :], op=mybir.AluOpType.mult)
                        else:
                            nc.scalar.activation(
                                out=sqt[:, :], in_=out_sb[:, t, :],
                                func=mybir.ActivationFunctionType.Square)
                        nc.tensor.matmul(
                            out=sum_ps[:, :], lhsT=ones[:, :],
                            rhs=out_sb[:, t, :],
                            start=(t == 0), stop=(t == Tb - 1),
                        )
                        nc.tensor.matmul(
                            out=sq_ps[:, :], lhsT=ones[:, :], rhs=sqt[:, :],
                            start=(t == 0), stop=(t == Tb - 1),
                        )
            st0 = bnspool.tile([1, 64], F32, tag="st0")
            st1 = bnspool.tile([1, 64], F32, tag="st1")
            nc.vector.tensor_copy(out=st0[:, :], in_=sum_ps[:, :])
            nc.vector.tensor_copy(out=st1[:, :], in_=sq_ps[:, :])
            nc.sync.dma_start(out=cc_in[0:1, :], in_=st0[:, :])
            nc.sync.dma_start(out=cc_in[1:2, :], in_=st1[:, :])
            nc.gpsimd.collective_compute(
                "AllReduce",
                mybir.AluOpType.add,
                ins=[cc_in[:, :]],
                outs=[cc_out[:, :]],
                replica_groups=[list(range(n_cores))],
            )
            gs0 = bnspool.tile([1, 64], F32, tag="gs0")
            gs1 = bnspool.tile([1, 64], F32, tag="gs1")
            nc.sync.dma_start(out=gs0[:, :], in_=cc_out[0:1, :])
            nc.sync.dma_start(out=gs1[:, :], in_=cc_out[1:2, :])
            gam_t = bnspool.tile([1, 64], F32, tag="gam")
            bet_t = bnspool.tile([1, 64], F32, tag="bet")
            nc.sync.dma_start(out=gam_t[:, :], in_=gb[0:1, :])
            nc.sync.dma_start(out=bet_t[:, :], in_=gb[1:2, :])

            inv_n = 1.0 / float(n_out)
            mean_t = bnspool.tile([1, 64], F32, tag="mean")
            ex2_t = bnspool.tile([1, 64], F32, tag="ex2")
            var_t = bnspool.tile([1, 64], F32, tag="var")
            sd_t = bnspool.tile([1, 64], F32, tag="sd")
            rs_t = bnspool.tile([1, 64], F32, tag="rs")
            a_t = bnspool.tile([1, 64], F32, tag="a")
            b_t = bnspool.tile([1, 64], F32, tag="b")
            nc.vector.tensor_scalar_mul(mean_t[:, :], gs0[:, :], inv_n)
            nc.vector.tensor_scalar_mul(ex2_t[:, :], gs1[:, :], inv_n)
            nc.vector.tensor_tensor(
                out=var_t[:, :], in0=mean_t[:, :], in1=mean_t[:, :],
                op=mybir.AluOpType.mult)
            nc.vector.tensor_tensor(
                out=var_t[:, :], in0=ex2_t[:, :], in1=var_t[:, :],
                op=mybir.AluOpType.subtract)
            nc.vector.tensor_scalar_add(var_t[:, :], var_t[:, :], BN_EPS)
            nc.scalar.activation(
                out=sd_t[:, :], in_=var_t[:, :],
                func=mybir.ActivationFunctionType.Sqrt)
            nc.vector.reciprocal(out=rs_t[:, :], in_=sd_t[:, :])
            nc.vector.tensor_tensor(
                out=a_t[:, :], in0=gam_t[:, :], in1=rs_t[:, :],
                op=mybir.AluOpType.mult)
            nc.vector.tensor_tensor(
                out=b_t[:, :], in0=mean_t[:, :], in1=a_t[:, :],
                op=mybir.AluOpType.mult)
            nc.vector.tensor_tensor(
                out=b_t[:, :], in0=bet_t[:, :], in1=b_t[:, :],
                op=mybir.AluOpType.subtract)
            # broadcast [1,64] -> [128,64] via PE (ones[1,128]^T @ row)
            ones_row = bnspool.tile([1, 128], F32, tag="ones_row")
            nc.vector.memset(ones_row[:, :], 1.0)
            a_full = bnspool.tile([128, 64], BF16, tag="afull")
            b_full = bnspool.tile([128, 64], BF16, tag="bfull")
            ab_ps = bnps.tile([128, 64], F32, tag="abps")
            nc.tensor.matmul(
                out=ab_ps[:, :], lhsT=ones_row[:, :], rhs=a_t[:, :],
                start=True, stop=True)
            nc.vector.tensor_copy(out=a_full[:, :], in_=ab_ps[:, :])
            nc.tensor.matmul(
                out=ab_ps[:, :], lhsT=ones_row[:, :], rhs=b_t[:, :],
                start=True, stop=True)
            nc.vector.tensor_copy(out=b_full[:, :], in_=ab_ps[:, :])
            # normalize in bf16, relu converts to f32 staging, write chunks
            with tc.tile_pool(name="bny", bufs=2) as ypool:
                for c0 in range(0, Tb, CH):
                    cc = min(CH, Tb - c0)
                    stage = ypool.tile([128, CH, 64], BF16, tag="stage")
                    for t in range(c0, c0 + cc):
                        nc.vector.tensor_tensor(
                            out=out_sb[:, t, :], in0=out_sb[:, t, :],
                            in1=a_full[:, :], op=mybir.AluOpType.mult)
                        nc.vector.tensor_tensor(
                            out=out_sb[:, t, :], in0=out_sb[:, t, :],
                            in1=b_full[:, :], op=mybir.AluOpType.add)
                        if t % 3 == 0:  # relu split: keep Act the short pole
                            nc.vector.tensor_scalar_max(
                                stage[:, t - c0, :], out_sb[:, t, :], 0.0)
                        else:
                            nc.scalar.activation(
                                out=stage[:, t - c0, :], in_=out_sb[:, t, :],
                                func=mybir.ActivationFunctionType.Relu)
                    nc.sync.dma_start(
                        out=y[c0 * 128:(c0 + cc) * 128, :],
                        in_=stage[:, :cc, :])

    nc.compile()
    return nc


def _pack_sidx(svals):
    """[total_slots] int -> [128, total_slots//16] int16 wrapped/tiled."""
    cols = svals.reshape(-1, 16).T.astype(np.int16)  # [16, n/16]
    return np.tile(cols, (8, 1))


def _prepare(feats, W, gamma, beta, in_map, out_map, n_out, n_cores=8,
             *_ignored):
    """Host prep shared by kernel() and tests. Returns (nc, in_maps, plan)."""
    n_out = int(n_out)
    K, Cin, Cout = W.shape
    assert Cin == 64 and Cout == 64
    in_map = np.asarray(in_map, dtype=np.int64)
    out_map = np.asarray(out_map, dtype=np.int64)
    feats = np.asarray(feats, dtype=np.float32)
    W = np.asarray(W, dtype=np.float32)

    plan, per_core = _route(in_map, out_map, n_out, n_cores)

    featsT = np.ascontiguousarray(
        feats.T.astype(ml_dtypes.bfloat16))          # [64, N_in]
    featsT_pad = np.concatenate(
        [featsT, np.zeros((64, 1), dtype=ml_dtypes.bfloat16)], axis=1)

    wt = np.ascontiguousarray(
        W.transpose(1, 0, 2).reshape(64, K * 64).astype(ml_dtypes.bfloat16))

    gb = np.stack([np.asarray(gamma, np.float32),
                   np.asarray(beta, np.float32)])

    nc = _build(plan, n_out, n_cores)
    in_maps = []
    for c in range(n_cores):
        gvals, svals, _ = per_core[c]
        slabt = featsT_pad[:, gvals]                 # -1 -> zero column
        in_maps.append(dict(slabt=np.ascontiguousarray(slabt), wt=wt,
                            sidx=_pack_sidx(svals), gb=gb))
    return nc, in_maps, plan


def kernel(feats, W, gamma, beta, in_map, out_map, n_out):
    from concourse.bass_utils import run_bass_kernel_spmd

    n_cores = 8
    nc, in_maps, plan = _prepare(
        feats, W, gamma, beta, in_map, out_map, n_out, n_cores)
    res = run_bass_kernel_spmd(nc, in_maps, list(range(n_cores)))
    rows = plan["rows_per_core"]
    out = np.concatenate(
        [res.results[c]["y"][:rows] for c in range(n_cores)], axis=0)
    return out.astype(np.float32)


# revision 61
# speedup vs baseline: 1.1766x; 1.0882x over previous
"""Trainium2 Bass kernel for nn_BasicDeconvolutionBlock.

Reference computation:
    gathered = feats[in_map]                         # [K, M, Cin]
    contrib  = einsum('kmc,kcd->kmd', gathered, W)   # [K, M, Cout]
    out      = zeros([n_out, Cout]).at[out_map].add(contrib)
    y        = relu(batchnorm(out))                  # batch stats over n_out rows

Strategy (8 NeuronCores, SPMD):
  - Host routes each (k, m) pair to the core owning its output row
    (row blocks of n_out/8) and lowers the gather to im2col: a per-core
    channel-major slab slabT[64, slots] (bf16) holding feats rows in
    k-major slot order, streamed to SBUF with large contiguous DMAs.
  - GEMM: per 128-slot tile (single k per tile), matmul(lhsT=slab tile
    [64ch,128slots], rhs=W[k][64ch,64]) -> PSUM [128slots,64] f32;
    PSUM->SBUF bf16 drains alternate between DVE and Activation.
  - Scatter: gpsimd dma_scatter_add (CCE-add, int16 idx, bf16 payload
    at 256B row stride) accumulates contributions into one of NBANKS=3
    HBM banks, rotating PER SEGMENT so nearby calls have no WAW hazard
    and the desc-gen / DMA / sem-prop chain pipelines ~3 deep.
    Duplicate output rows inside one call race in hardware, so the host
    spaces a row's occurrences ~cnt/h apart inside each k group,
    swap-fixes residual in-segment duplicates, and halves the rare
    segments whose duplicates are pigeonhole-stuck.  Scatter calls are
    12*128 = 1536 indices; the SWDGE descriptor ring is enlarged via
    dynamic_dma_scratch_size=73728 (the stock 16 KiB ring is what
    capped calls at ~896 indices and wedged beyond it).
  - BN: fold the banks chunk-wise (strided 64-of-128-col reads),
    ones-matmul row sums + Square sums interleaved with the fold,
    [2,64] AllReduce across the 8 cores, normalize + ReLU into a bf16
    staging tile, write y (bf16); the host casts to f32.
"""

import os
import sys

import numpy as np

sys.path.insert(0, "/opt/trn_rl_repo")

import ml_dtypes  # noqa: E402

from concourse import bacc, bass, mybir  # noqa: E402
import concourse.tile as tile  # noqa: E402

BN_EPS = 1e-5
SEG_TILES = int(os.environ.get("DECONV_SEG_TILES", "12"))
SUPER_SEGS = int(os.environ.get("DECONV_SUPER_SEGS", "2"))
NBANKS = int(os.environ.get("DECONV_NBANKS", "3"))
# SWDGE descriptor-ring carveout (SBUF bytes; ring = bytes/16 descs).
# Must hold the in-flight scatter calls (NBANKS x SEG_TILES x 128 descs).
# The stock 16384 ring is why calls used to be capped at 896 indices.
DMA_SCRATCH = int(os.environ.get("DECONV_SCRATCH", "73728"))
F32 = mybir.dt.float32
BF16 = mybir.dt.bfloat16
I16 = mybir.dt.int16


def _roundup(x, m):
    return (x + m - 1) // m * m


def _order_group(rows):
    """Slot order for one (core, k) group: spread a row's occurrences
    ~cnt/h apart so same-call duplicates are rare. Returns a permutation
    of range(len(rows))."""
    n = len(rows)
    if n == 0:
        return np.empty(0, dtype=np.int64)
    order = np.argsort(rows, kind="stable")
    sr = rows[order]
    first = np.ones(n, dtype=bool)
    first[1:] = sr[1:] != sr[:-1]
    grp = np.cumsum(first) - 1                    # rank of unique row
    grp_start = np.maximum.accumulate(np.where(first, np.arange(n), 0))
    occ = np.arange(n) - grp_start                # occurrence index j
    # occurrence count h per element
    cnt_per_grp = np.bincount(grp)
    h = cnt_per_grp[grp]
    nuniq = cnt_per_grp.size
    key = (occ + grp / max(nuniq, 1)) / h
    final = np.argsort(key, kind="stable")
    return order[final]


def _fix_conflicts(svals, gvals, seg_bounds, group_bounds, dump_row):
    """Ensure no duplicate (non-dump) rows within any segment by swapping
    slots within their k-group. svals/gvals modified in place."""
    nslots = len(svals)
    nseg = len(seg_bounds) - 1
    seg_of = np.zeros(nslots, dtype=np.int64)
    for s in range(nseg):
        seg_of[seg_bounds[s]:seg_bounds[s + 1]] = s
    grp_of = np.zeros(nslots, dtype=np.int64)
    for g in range(len(group_bounds) - 1):
        grp_of[group_bounds[g]:group_bounds[g + 1]] = g

    # per-seg row sets
    seg_sets = [set() for _ in range(nseg)]
    conflicts = []
    failed = set()
    is_conflict = np.zeros(nslots, dtype=bool)
    for i in range(nslots):
        r = svals[i]
        if r == dump_row:
            continue
        ss = seg_sets[seg_of[i]]
        if r in ss:
            conflicts.append(i)
            is_conflict[i] = True
        else:
            ss.add(r)
    rng = np.random.default_rng(0)
    for i in conflicts:
        r = int(svals[i])
        g = grp_of[i]
        lo, hi = group_bounds[g], group_bounds[g + 1]
        placed = False
        cands = list(rng.integers(lo, hi, size=200)) + list(range(lo, hi))
        for j in cands:
            j = int(j)
            sj = seg_of[j]
            if sj == seg_of[i] or is_conflict[j]:
                continue
            rj = int(svals[j])
            # after swap: r goes to seg sj, rj comes to seg of i
            if r in seg_sets[sj]:
                continue
            if rj != dump_row:
                if rj == r or rj in seg_sets[seg_of[i]]:
                    continue
            # apply swap
            si = seg_of[i]
            if rj != dump_row:
                seg_sets[sj].discard(rj)
                seg_sets[si].add(rj)
            seg_sets[sj].add(r)
            svals[i], svals[j] = svals[j], svals[i]
            gvals[i], gvals[j] = gvals[j], gvals[i]
            is_conflict[i] = False
            placed = True
            break
        if not placed:
            failed.add(int(seg_of[i]))
    return len(conflicts), failed


def _route(in_map, out_map, n_out, n_cores):
    """Host-side routing. Returns plan + per-core slot arrays
    (gvals: feats row per slot or -1; svals: local out row per slot)."""
    K, M = in_map.shape
    rows_per_core = n_out // n_cores
    assert rows_per_core * n_cores == n_out
    acc_rows = _roundup(rows_per_core, 128)
    dump_row = acc_rows
    acc_total = acc_rows + 128

    in_flat = in_map.ravel().astype(np.int64)
    out_flat = out_map.ravel().astype(np.int64)
    k_idx = np.repeat(np.arange(K, dtype=np.int64), M)
    core = out_flat // rows_per_core
    row_local = (out_flat - core * rows_per_core).astype(np.int64)

    # per (core, k) counts -> shared caps
    counts = np.zeros((n_cores, K), dtype=np.int64)
    np.add.at(counts, (core, k_idx), 1)
    caps = _roundup(counts.max(axis=0), 128)  # [K]
    group_bounds = np.concatenate([[0], np.cumsum(caps)])
    total_slots = int(group_bounds[-1])

    seg_slots = SEG_TILES * 128
    seg_bounds = list(range(0, total_slots, seg_slots)) + [total_slots]
    if seg_bounds[-1] == seg_bounds[-2]:
        seg_bounds.pop()

    # iteratively split segments whose duplicate conflicts can't be
    # swap-fixed (rare: rows occupying every window their group spans)
    for _ in range(8):
        per_core = []
        all_failed = set()
        for c in range(n_cores):
            gvals = np.full(total_slots, -1, dtype=np.int64)
            svals = np.full(total_slots, dump_row, dtype=np.int64)
            sel_c = core == c
            for k in range(K):
                sel = np.nonzero(sel_c & (k_idx == k))[0]
                rows_k = row_local[sel]
                perm = _order_group(rows_k)
                g0 = group_bounds[k]
                n = len(sel)
                gvals[g0:g0 + n] = in_flat[sel][perm]
                svals[g0:g0 + n] = rows_k[perm]
            nfix, failed = _fix_conflicts(svals, gvals, seg_bounds,
                                          group_bounds, dump_row)
            per_core.append((gvals, svals, nfix))
            all_failed |= failed
        if not all_failed:
            break
        new_bounds = []
        for s in range(len(seg_bounds) - 1):
            a, b = seg_bounds[s], seg_bounds[s + 1]
            new_bounds.append(a)
            if s in all_failed and b - a >= 256:
                new_bounds.append(a + (b - a) // 256 * 128)
        new_bounds.append(seg_bounds[-1])
        seg_bounds = new_bounds
    else:
        raise RuntimeError("segment splitting did not converge")

    # tile -> k map
    ntiles = total_slots // 128
    tile_k = np.zeros(ntiles, dtype=np.int64)
    for k in range(K):
        tile_k[group_bounds[k] // 128:group_bounds[k + 1] // 128] = k

    # Device y layout: the SBUF-dst scatter ucode places token idx
    # (se=B//2, p=idx%128, parity=B%2) at flat position
    # T = se%(G/2) + (G/2)*p + (G/2)*128*(se//(G/2)) within its parity
    # tensor (reverse-engineered empirically; exact on all rows).
    G = (acc_rows // 128) // 2
    H = G // 2
    n = np.arange(acc_rows)
    blk, part = n // 128, n % 128
    se, par = blk // 2, blk % 2
    y_perm = par * (G * 128) + (se % H) + H * part + H * 128 * (se // H)

    plan = dict(
        K=K, rows_per_core=rows_per_core, acc_rows=acc_rows,
        acc_total=acc_total, dump_row=dump_row,
        total_slots=total_slots, ntiles=ntiles, tile_k=tile_k,
        seg_bounds=seg_bounds, seg_slots=seg_slots, y_perm=y_perm,
    )
    return plan, per_core


def _build(plan, n_out, n_cores):
    """Trace the Bass program. Returns nc."""
    nc = bacc.Bacc("TRN2", target_bir_lowering=False, debug=False,
                   dynamic_dma_scratch_size=DMA_SCRATCH)

    K = plan["K"]
    acc_rows, acc_total = plan["acc_rows"], plan["acc_total"]
    total_slots = plan["total_slots"]
    tile_k = plan["tile_k"]
    seg_bounds = plan["seg_bounds"]
    nseg = len(seg_bounds) - 1
    Cout = 64

    slabt = nc.dram_tensor("slabt", [64, total_slots], BF16,
                           kind="ExternalInput")
    wt = nc.dram_tensor("wt", [64, K * Cout], BF16, kind="ExternalInput")
    sidx = nc.dram_tensor("sidx", [128, total_slots // 16], I16,
                          kind="ExternalInput")
    gb = nc.dram_tensor("gb", [2, Cout], F32, kind="ExternalInput")
    cc_in = nc.dram_tensor("cc_in", [2, Cout], F32)
    cc_out = nc.dram_tensor("cc_out", [2, Cout], F32, addr_space="Shared")
    # y is written parity-major: dev row = ((B%2)*G + B//2)*128 + part
    # for true row B*128+part; the host un-permutes (plan['y_perm']).
    y = nc.dram_tensor("y", [acc_rows, Cout], BF16, kind="ExternalOutput")

    Tb = acc_rows // 128  # 128-row blocks
    G = Tb // 2           # g-slots per parity (SBUF scatter layout)
    GD = G + 1            # +1 g-slot for the dump row (even parity)

    # super-segment layout: SUPER_SEGS segments per slab load
    supers = []
    s = 0
    while s < nseg:
        e = min(s + SUPER_SEGS, nseg)
        supers.append((s, e))
        s = e

    with tile.TileContext(nc) as tc:
        with tc.tile_pool(name="acc", bufs=1) as accpool:
            # SBUF accumulator banks: (even, odd) parity pair per bank.
            # Token idx -> partition idx%128, free offset (idx//256)*64,
            # even/odd tensor by (idx//128)%2.  Zeroed by memset (no DMA).
            acc_eo = []
            for b in range(NBANKS):
                e = accpool.tile([128, GD * Cout], BF16, tag=f"acce{b}")
                o = accpool.tile([128, GD * Cout], BF16, tag=f"acco{b}")
                nc.vector.memset(e[:, :], 0.0)
                nc.vector.memset(o[:, :], 0.0)
                acc_eo.append((e, o))
            _build_body(nc, tc, plan, n_out, n_cores, acc_eo,
                        slabt, wt, sidx, gb, cc_in, cc_out, y,
                        supers, seg_bounds, tile_k, Tb, G, Cout)

    nc.compile()
    return nc


def _build_body(nc, tc, plan, n_out, n_cores, acc_eo, slabt, wt, sidx,
                gb, cc_in, cc_out, y, supers, seg_bounds, tile_k, Tb, G,
                Cout):
        K = plan["K"]
        GD = G + 1
        with (
            tc.tile_pool(name="const", bufs=1) as cpool,
            tc.tile_pool(name="slab", bufs=4) as slabpool,
            tc.tile_pool(name="oslab", bufs=8) as opool,
            tc.tile_pool(name="sixp", bufs=4) as sixpool,
            tc.tile_pool(name="psum", bufs=8, space="PSUM") as pspool,
        ):
            w_sb = cpool.tile([64, K * Cout], BF16, tag="w")
            nc.sync.dma_start(out=w_sb[:, :], in_=wt[:, :])
            for (s0seg, s1seg) in supers:
                a = seg_bounds[s0seg]
                b = seg_bounds[s1seg]
                ns_sup = b - a
                g = slabpool.tile([64, SUPER_SEGS * plan["seg_slots"]],
                                  BF16, tag="g")
                nc.sync.dma_start(out=g[:, :ns_sup], in_=slabt[:, a:b])
                si_t = sixpool.tile(
                    [128, SUPER_SEGS * plan["seg_slots"] // 16], I16,
                    tag="si")
                nc.sync.dma_start(
                    out=si_t[:, :ns_sup // 16],
                    in_=sidx[:, a // 16:b // 16],
                )
                for seg in range(s0seg, s1seg):
                    sa = seg_bounds[seg]
                    sb = seg_bounds[seg + 1]
                    ns = sb - sa
                    ntile = ns // 128
                    oslab = opool.tile([128, SEG_TILES, Cout], BF16,
                                       tag="oslab")
                    for t in range(ntile):
                        col = (sa - a) + t * 128
                        k = int(tile_k[sa // 128 + t])
                        ps = pspool.tile([128, Cout], F32, tag="ps")
                        nc.tensor.matmul(
                            out=ps[:, :],
                            lhsT=g[:, col:col + 128],
                            rhs=w_sb[:, k * Cout:(k + 1) * Cout],
                            start=True, stop=True,
                        )
                        if t % 2 == 0:  # split PSUM drains across DVE/Act
                            nc.vector.tensor_copy(
                                out=oslab[:, t, :], in_=ps[:, :])
                        else:
                            nc.scalar.activation(
                                out=oslab[:, t, :], in_=ps[:, :],
                                func=mybir.ActivationFunctionType.Copy)
                    acc_e, acc_o = acc_eo[seg % NBANKS]
                    nc.gpsimd.dma_scatter_add(
                        out_ap=acc_e[:, :],
                        in_ap=oslab[:, :ntile, :],
                        idxs_ap=si_t[:, (sa - a) // 16:(sb - a) // 16],
                        num_idxs=ns,
                        num_idxs_reg=ns,
                        elem_size=Cout,
                        sbuf_tokens_per_rank=128,
                        parity_reg=int(os.environ.get("DECONV_PARITY", "0")),
                        out_ap_other=acc_o[:, :],
                    )

        # ---- BN phase ----
        with (
            tc.tile_pool(name="bn", bufs=1) as bnpool,
            tc.tile_pool(name="bns", bufs=4) as bnspool,
            tc.tile_pool(name="bnp", bufs=2, space="PSUM") as bnps,
        ):
            ones = bnpool.tile([128, 1], BF16, tag="ones")
            nc.vector.memset(ones[:, :], 1.0)
            sum_ps = bnps.tile([1, 64], F32, tag="sum")
            sq_ps = bnps.tile([1, 64], F32, tag="sq")
            # fold banks 1.. into bank 0 in place (per parity, skip dump g)
            fold_e, fold_o = acc_eo[0]
            for b in range(1, NBANKS):
                nc.vector.tensor_tensor(
                    out=fold_e[:, :G * 64], in0=fold_e[:, :G * 64],
                    in1=acc_eo[b][0][:, :G * 64], op=mybir.AluOpType.add)
                nc.vector.tensor_tensor(
                    out=fold_o[:, :G * 64], in0=fold_o[:, :G * 64],
                    in1=acc_eo[b][1][:, :G * 64], op=mybir.AluOpType.add)
            folds = (fold_e, fold_o)
            with tc.tile_pool(name="bnsq", bufs=4) as sqpool:
                for t in range(Tb):
                    src = folds[t % 2][:, (t // 2) * 64:(t // 2 + 1) * 64]
                    sqt = sqpool.tile([128, 64], BF16, tag="sqt")
                    if t % 4 == 0:
                        nc.vector.tensor_tensor(
                            out=sqt[:, :], in0=src, in1=src,
                            op=mybir.AluOpType.mult)
                    else:
                        nc.scalar.activation(
                            out=sqt[:, :], in_=src,
                            func=mybir.ActivationFunctionType.Square)
                    nc.tensor.matmul(
                        out=sum_ps[:, :], lhsT=ones[:, :], rhs=src,
                        start=(t == 0), stop=(t == Tb - 1),
                    )
                    nc.tensor.matmul(
                        out=sq_ps[:, :], lhsT=ones[:, :], rhs=sqt[:, :],
                        start=(t == 0), stop=(t == Tb - 1),
                    )
            st0 = bnspool.tile([1, 64], F32, tag="st0")
            st1 = bnspool.tile([1, 64], F32, tag="st1")
            nc.vector.tensor_copy(out=st0[:, :], in_=sum_ps[:, :])
            nc.vector.tensor_copy(out=st1[:, :], in_=sq_ps[:, :])
            nc.sync.dma_start(out=cc_in[0:1, :], in_=st0[:, :])
            nc.sync.dma_start(out=cc_in[1:2, :], in_=st1[:, :])
            nc.gpsimd.collective_compute(
                "AllReduce",
                mybir.AluOpType.add,
                ins=[cc_in[:, :]],
                outs=[cc_out[:, :]],
                replica_groups=[list(range(n_cores))],
            )
            gs0 = bnspool.tile([1, 64], F32, tag="gs0")
            gs1 = bnspool.tile([1, 64], F32, tag="gs1")
            nc.sync.dma_start(out=gs0[:, :], in_=cc_out[0:1, :])
            nc.sync.dma_start(out=gs1[:, :], in_=cc_out[1:2, :])
            gam_t = bnspool.tile([1, 64], F32, tag="gam")
            bet_t = bnspool.tile([1, 64], F32, tag="bet")
            nc.sync.dma_start(out=gam_t[:, :], in_=gb[0:1, :])
            nc.sync.dma_start(out=bet_t[:, :], in_=gb[1:2, :])

            inv_n = 1.0 / float(n_out)
            mean_t = bnspool.tile([1, 64], F32, tag="mean")
            ex2_t = bnspool.tile([1, 64], F32, tag="ex2")
            var_t = bnspool.tile([1, 64], F32, tag="var")
            sd_t = bnspool.tile([1, 64], F32, tag="sd")
            rs_t = bnspool.tile([1, 64], F32, tag="rs")
            a_t = bnspool.tile([1, 64], F32, tag="a")
            b_t = bnspool.tile([1, 64], F32, tag="b")
            nc.vector.tensor_scalar_mul(mean_t[:, :], gs0[:, :], inv_n)
            nc.vector.tensor_scalar_mul(ex2_t[:, :], gs1[:, :], inv_n)
            nc.vector.tensor_tensor(
                out=var_t[:, :], in0=mean_t[:, :], in1=mean_t[:, :],
                op=mybir.AluOpType.mult)
            nc.vector.tensor_tensor(
                out=var_t[:, :], in0=ex2_t[:, :], in1=var_t[:, :],
                op=mybir.AluOpType.subtract)
            nc.vector.tensor_scalar_add(var_t[:, :], var_t[:, :], BN_EPS)
            nc.scalar.activation(
                out=sd_t[:, :], in_=var_t[:, :],
                func=mybir.ActivationFunctionType.Sqrt)
            nc.vector.reciprocal(out=rs_t[:, :], in_=sd_t[:, :])
            nc.vector.tensor_tensor(
                out=a_t[:, :], in0=gam_t[:, :], in1=rs_t[:, :],
                op=mybir.AluOpType.mult)
            nc.vector.tensor_tensor(
                out=b_t[:, :], in0=mean_t[:, :], in1=a_t[:, :],
                op=mybir.AluOpType.mult)
            nc.vector.tensor_tensor(
                out=b_t[:, :], in0=bet_t[:, :], in1=b_t[:, :],
                op=mybir.AluOpType.subtract)
            # broadcast [1,64] -> [128,64] via PE (ones[1,128]^T @ row)
            ones_row = bnspool.tile([1, 128], F32, tag="ones_row")
            nc.vector.memset(ones_row[:, :], 1.0)
            a_full = bnspool.tile([128, 64], BF16, tag="afull")
            b_full = bnspool.tile([128, 64], BF16, tag="bfull")
            ab_ps = bnps.tile([128, 64], F32, tag="abps")
            nc.tensor.matmul(
                out=ab_ps[:, :], lhsT=ones_row[:, :], rhs=a_t[:, :],
                start=True, stop=True)
            nc.vector.tensor_copy(out=a_full[:, :], in_=ab_ps[:, :])
            nc.tensor.matmul(
                out=ab_ps[:, :], lhsT=ones_row[:, :], rhs=b_t[:, :],
                start=True, stop=True)
            nc.vector.tensor_copy(out=b_full[:, :], in_=ab_ps[:, :])
            # normalize in place (bf16), relu into staging, write y in
            # parity-major chunks; the host un-permutes rows (y_perm)
            CH = 49
            with tc.tile_pool(name="bny", bufs=2) as ypool:
                for p in range(2):
                    for g0 in range(0, G, CH):
                        gc = min(CH, G - g0)
                        stage = ypool.tile([128, CH, 64], BF16, tag="stage")
                        for gg in range(g0, g0 + gc):
                            src = folds[p][:, gg * 64:(gg + 1) * 64]
                            nc.vector.tensor_tensor(
                                out=src, in0=src, in1=a_full[:, :],
                                op=mybir.AluOpType.mult)
                            nc.vector.tensor_tensor(
                                out=src, in0=src, in1=b_full[:, :],
                                op=mybir.AluOpType.add)
                            if gg % 3 == 0:  # relu split across DVE/Act
                                nc.vector.tensor_scalar_max(
                                    stage[:, gg - g0, :], src, 0.0)
                            else:
                                nc.scalar.activation(
                                    out=stage[:, gg - g0, :], in_=src,
                                    func=mybir.ActivationFunctionType.Relu)
                        r0 = (p * G + g0) * 128
                        nc.sync.dma_start(
                            out=y[r0:r0 + gc * 128, :],
                            in_=stage[:, :gc, :])


def _pack_sidx(svals):
    """[total_slots] int -> [128, total_slots//16] int16 wrapped/tiled."""
    cols = svals.reshape(-1, 16).T.astype(np.int16)  # [16, n/16]
    return np.tile(cols, (8, 1))


def _prepare(feats, W, gamma, beta, in_map, out_map, n_out, n_cores=8,
             *_ignored):
    """Host prep shared by kernel() and tests. Returns (nc, in_maps, plan)."""
    n_out = int(n_out)
    K, Cin, Cout = W.shape
    assert Cin == 64 and Cout == 64
    in_map = np.asarray(in_map, dtype=np.int64)
    out_map = np.asarray(out_map, dtype=np.int64)
    feats = np.asarray(feats, dtype=np.float32)
    W = np.asarray(W, dtype=np.float32)

    plan, per_core = _route(in_map, out_map, n_out, n_cores)

    featsT = np.ascontiguousarray(
        feats.T.astype(ml_dtypes.bfloat16))          # [64, N_in]
    featsT_pad = np.concatenate(
        [featsT, np.zeros((64, 1), dtype=ml_dtypes.bfloat16)], axis=1)

    wt = np.ascontiguousarray(
        W.transpose(1, 0, 2).reshape(64, K * 64).astype(ml_dtypes.bfloat16))

    gb = np.stack([np.asarray(gamma, np.float32),
                   np.asarray(beta, np.float32)])

    nc = _build(plan, n_out, n_cores)
    in_maps = []
    for c in range(n_cores):
        gvals, svals, _ = per_core[c]
        slabt = featsT_pad[:, gvals]                 # -1 -> zero column
        in_maps.append(dict(slabt=np.ascontiguousarray(slabt), wt=wt,
                            sidx=_pack_sidx(svals), gb=gb))
    return nc, in_maps, plan


def kernel(feats, W, gamma, beta, in_map, out_map, n_out):
    from concourse.bass_utils import run_bass_kernel_spmd

    n_cores = 8
    nc, in_maps, plan = _prepare(
        feats, W, gamma, beta, in_map, out_map, n_out, n_cores)
    res = run_bass_kernel_spmd(nc, in_maps, list(range(n_cores)))
    rows = plan["rows_per_core"]
    perm = plan["y_perm"]
    out = np.concatenate(
        [res.results[c]["y"][perm][:rows] for c in range(n_cores)], axis=0)
    return out.astype(np.float32)


# revision 67
# speedup vs baseline: 1.3286x; 1.1292x over previous
"""Trainium2 Bass kernel for nn_BasicDeconvolutionBlock.

Reference computation:
    gathered = feats[in_map]                         # [K, M, Cin]
    contrib  = einsum('kmc,kcd->kmd', gathered, W)   # [K, M, Cout]
    out      = zeros([n_out, Cout]).at[out_map].add(contrib)
    y        = relu(batchnorm(out))                  # batch stats over n_out rows

Strategy (8 NeuronCores, SPMD):
  - Host routes each (k, m) pair to the core owning its output row
    (row blocks of n_out/8) and lowers the gather to im2col: a per-core
    channel-major slab slabT[64, slots] (bf16) holding feats rows in
    k-major slot order, streamed to SBUF with large contiguous DMAs.
  - GEMM: per 128-slot tile (single k per tile), matmul(lhsT=slab tile
    [64ch,128slots], rhs=W[k][64ch,64]) -> PSUM [128slots,64] f32;
    PSUM->SBUF bf16 drains alternate between DVE and Activation.
  - Scatter: gpsimd dma_scatter_add in SBUF-destination parity-split
    mode (CCE-add, int16 idx, bf16 payload) accumulates directly into
    SBUF (even,odd) accumulator pairs — no HBM zero-init or readback;
    zeroing is an on-chip memset.  NBANKS=3 pairs rotate PER SEGMENT so
    nearby calls have no WAW hazard and the desc-gen / DMA / sem-prop
    chain pipelines ~3 deep.  Duplicate output rows inside one call
    race in hardware, so the host spaces a row's occurrences ~cnt/h
    apart inside each k group, swap-fixes residual in-segment
    duplicates, and halves the rare segments whose duplicates are
    pigeonhole-stuck.  Scatter calls are 12*128 = 1536 indices; the
    SWDGE descriptor ring is enlarged via
    dynamic_dma_scratch_size=73728 (the stock 16 KiB ring is what
    capped calls at ~896 indices and wedged beyond it).
  - BN: fold banks 1.. into bank 0 in place (flat bf16 DVE adds),
    ones-matmul row sums + Square sums per 64-col group, [2,64]
    AllReduce across the 8 cores, normalize + ReLU into bf16 staging,
    write y parity-major.  The ucode's SBUF token layout
    (T = se%49 + 49*p + 6272*(se//49) per parity, se=row//256,
    p=row%128 — reverse-engineered empirically) is undone by a host-
    side row permutation (plan['y_perm']); host casts bf16->f32.
"""

import os
import sys

import numpy as np

sys.path.insert(0, "/opt/trn_rl_repo")

import ml_dtypes  # noqa: E402

from concourse import bacc, bass, mybir  # noqa: E402
import concourse.tile as tile  # noqa: E402

BN_EPS = 1e-5
SEG_TILES = int(os.environ.get("DECONV_SEG_TILES", "12"))
SUPER_SEGS = int(os.environ.get("DECONV_SUPER_SEGS", "2"))
NBANKS = int(os.environ.get("DECONV_NBANKS", "3"))
# SWDGE descriptor-ring carveout (SBUF bytes; ring = bytes/16 descs).
# Must hold the in-flight scatter calls (NBANKS x SEG_TILES x 128 descs).
# The stock 16384 ring is why calls used to be capped at 896 indices.
DMA_SCRATCH = int(os.environ.get("DECONV_SCRATCH", "73728"))
F32 = mybir.dt.float32
BF16 = mybir.dt.bfloat16
I16 = mybir.dt.int16


def _roundup(x, m):
    return (x + m - 1) // m * m


def _order_group(rows):
    """Slot order for one (core, k) group: spread a row's occurrences
    ~cnt/h apart so same-call duplicates are rare. Returns a permutation
    of range(len(rows))."""
    n = len(rows)
    if n == 0:
        return np.empty(0, dtype=np.int64)
    order = np.argsort(rows, kind="stable")
    sr = rows[order]
    first = np.ones(n, dtype=bool)
    first[1:] = sr[1:] != sr[:-1]
    grp = np.cumsum(first) - 1                    # rank of unique row
    grp_start = np.maximum.accumulate(np.where(first, np.arange(n), 0))
    occ = np.arange(n) - grp_start                # occurrence index j
    # occurrence count h per element
    cnt_per_grp = np.bincount(grp)
    h = cnt_per_grp[grp]
    nuniq = cnt_per_grp.size
    key = (occ + grp / max(nuniq, 1)) / h
    final = np.argsort(key, kind="stable")
    return order[final]


def _fix_conflicts(svals, gvals, seg_bounds, group_bounds, dump_row):
    """Ensure no duplicate (non-dump) rows within any segment by swapping
    slots within their k-group. svals/gvals modified in place."""
    nslots = len(svals)
    nseg = len(seg_bounds) - 1
    seg_of = np.zeros(nslots, dtype=np.int64)
    for s in range(nseg):
        seg_of[seg_bounds[s]:seg_bounds[s + 1]] = s
    grp_of = np.zeros(nslots, dtype=np.int64)
    for g in range(len(group_bounds) - 1):
        grp_of[group_bounds[g]:group_bounds[g + 1]] = g

    # per-seg row sets
    seg_sets = [set() for _ in range(nseg)]
    conflicts = []
    failed = set()
    is_conflict = np.zeros(nslots, dtype=bool)
    for i in range(nslots):
        r = svals[i]
        if r == dump_row:
            continue
        ss = seg_sets[seg_of[i]]
        if r in ss:
            conflicts.append(i)
            is_conflict[i] = True
        else:
            ss.add(r)
    rng = np.random.default_rng(0)
    for i in conflicts:
        r = int(svals[i])
        g = grp_of[i]
        lo, hi = group_bounds[g], group_bounds[g + 1]
        placed = False
        cands = list(rng.integers(lo, hi, size=200)) + list(range(lo, hi))
        for j in cands:
            j = int(j)
            sj = seg_of[j]
            if sj == seg_of[i] or is_conflict[j]:
                continue
            rj = int(svals[j])
            # after swap: r goes to seg sj, rj comes to seg of i
            if r in seg_sets[sj]:
                continue
            if rj != dump_row:
                if rj == r or rj in seg_sets[seg_of[i]]:
                    continue
            # apply swap
            si = seg_of[i]
            if rj != dump_row:
                seg_sets[sj].discard(rj)
                seg_sets[si].add(rj)
            seg_sets[sj].add(r)
            svals[i], svals[j] = svals[j], svals[i]
            gvals[i], gvals[j] = gvals[j], gvals[i]
            is_conflict[i] = False
            placed = True
            break
        if not placed:
            failed.add(int(seg_of[i]))
    return len(conflicts), failed


def _route(in_map, out_map, n_out, n_cores):
    """Host-side routing. Returns plan + per-core slot arrays
    (gvals: feats row per slot or -1; svals: local out row per slot)."""
    K, M = in_map.shape
    rows_per_core = n_out // n_cores
    assert rows_per_core * n_cores == n_out
    acc_rows = _roundup(rows_per_core, 128)
    dump_row = acc_rows
    acc_total = acc_rows + 128

    in_flat = in_map.ravel().astype(np.int64)
    out_flat = out_map.ravel().astype(np.int64)
    k_idx = np.repeat(np.arange(K, dtype=np.int64), M)
    core = out_flat // rows_per_core
    row_local = (out_flat - core * rows_per_core).astype(np.int64)

    # per (core, k) counts -> shared caps
    counts = np.zeros((n_cores, K), dtype=np.int64)
    np.add.at(counts, (core, k_idx), 1)
    caps = _roundup(counts.max(axis=0), 128)  # [K]
    group_bounds = np.concatenate([[0], np.cumsum(caps)])
    total_slots = int(group_bounds[-1])

    seg_slots = SEG_TILES * 128
    seg_bounds = list(range(0, total_slots, seg_slots)) + [total_slots]
    if seg_bounds[-1] == seg_bounds[-2]:
        seg_bounds.pop()

    # iteratively split segments whose duplicate conflicts can't be
    # swap-fixed (rare: rows occupying every window their group spans)
    for _ in range(8):
        per_core = []
        all_failed = set()
        for c in range(n_cores):
            gvals = np.full(total_slots, -1, dtype=np.int64)
            svals = np.full(total_slots, dump_row, dtype=np.int64)
            sel_c = core == c
            for k in range(K):
                sel = np.nonzero(sel_c & (k_idx == k))[0]
                rows_k = row_local[sel]
                perm = _order_group(rows_k)
                g0 = group_bounds[k]
                n = len(sel)
                gvals[g0:g0 + n] = in_flat[sel][perm]
                svals[g0:g0 + n] = rows_k[perm]
            nfix, failed = _fix_conflicts(svals, gvals, seg_bounds,
                                          group_bounds, dump_row)
            per_core.append((gvals, svals, nfix))
            all_failed |= failed
        if not all_failed:
            break
        new_bounds = []
        for s in range(len(seg_bounds) - 1):
            a, b = seg_bounds[s], seg_bounds[s + 1]
            new_bounds.append(a)
            if s in all_failed and b - a >= 256:
                new_bounds.append(a + (b - a) // 256 * 128)
        new_bounds.append(seg_bounds[-1])
        seg_bounds = new_bounds
    else:
        raise RuntimeError("segment splitting did not converge")

    # tile -> k map
    ntiles = total_slots // 128
    tile_k = np.zeros(ntiles, dtype=np.int64)
    for k in range(K):
        tile_k[group_bounds[k] // 128:group_bounds[k + 1] // 128] = k

    # Device y layout: SBUF-dst scatter places true row B*128+p at
    # accumulator coord (partition=p, group se=B//2, parity=B%2); the
    # per-parity stage->y DMA writes partition-major (dev row p*G + se).
    G = (acc_rows // 128) // 2
    n = np.arange(acc_rows)
    blk, part = n // 128, n % 128
    se, par = blk // 2, blk % 2
    y_perm = par * (G * 128) + part * G + se

    plan = dict(
        K=K, rows_per_core=rows_per_core, acc_rows=acc_rows,
        acc_total=acc_total, dump_row=dump_row,
        total_slots=total_slots, ntiles=ntiles, tile_k=tile_k,
        seg_bounds=seg_bounds, seg_slots=seg_slots, y_perm=y_perm,
    )
    return plan, per_core


def _build(plan, n_out, n_cores):
    """Trace the Bass program. Returns nc."""
    nc = bacc.Bacc("TRN2", target_bir_lowering=False, debug=False,
                   dynamic_dma_scratch_size=DMA_SCRATCH)

    K = plan["K"]
    acc_rows, acc_total = plan["acc_rows"], plan["acc_total"]
    total_slots = plan["total_slots"]
    tile_k = plan["tile_k"]
    seg_bounds = plan["seg_bounds"]
    nseg = len(seg_bounds) - 1
    Cout = 64

    slabt = nc.dram_tensor("slabt", [64, total_slots], BF16,
                           kind="ExternalInput")
    wt = nc.dram_tensor("wt", [64, K * Cout], BF16, kind="ExternalInput")
    sidx = nc.dram_tensor("sidx", [128, total_slots // 16], I16,
                          kind="ExternalInput")
    gb = nc.dram_tensor("gb", [2, Cout], F32, kind="ExternalInput")
    cc_in = nc.dram_tensor("cc_in", [2, Cout], F32)
    cc_out = nc.dram_tensor("cc_out", [2, Cout], F32, addr_space="Shared")
    # y is written parity-major: dev row = ((B%2)*G + B//2)*128 + part
    # for true row B*128+part; the host un-permutes (plan['y_perm']).
    y = nc.dram_tensor("y", [acc_rows, Cout], BF16, kind="ExternalOutput")

    Tb = acc_rows // 128  # 128-row blocks
    G = Tb // 2           # g-slots per parity (SBUF scatter layout)
    GD = G + 1            # +1 g-slot for the dump row (even parity)

    # super-segment layout: SUPER_SEGS segments per slab load
    supers = []
    s = 0
    while s < nseg:
        e = min(s + SUPER_SEGS, nseg)
        supers.append((s, e))
        s = e

    with tile.TileContext(nc) as tc:
        with tc.tile_pool(name="acc", bufs=1) as accpool:
            # SBUF accumulator banks: (even, odd) parity pair per bank.
            # Token idx -> partition idx%128, free offset (idx//256)*64,
            # even/odd tensor by (idx//128)%2.  Zeroed by memset (no DMA).
            acc_eo = []
            for b in range(NBANKS):
                e = accpool.tile([128, GD * Cout], BF16, tag=f"acce{b}")
                o = accpool.tile([128, GD * Cout], BF16, tag=f"acco{b}")
                nc.vector.memset(e[:, :], 0.0)
                nc.vector.memset(o[:, :], 0.0)
                acc_eo.append((e, o))
            _build_body(nc, tc, plan, n_out, n_cores, acc_eo,
                        slabt, wt, sidx, gb, cc_in, cc_out, y,
                        supers, seg_bounds, tile_k, Tb, G, Cout)

    nc.compile()
    return nc


def _build_body(nc, tc, plan, n_out, n_cores, acc_eo, slabt, wt, sidx,
                gb, cc_in, cc_out, y, supers, seg_bounds, tile_k, Tb, G,
                Cout):
        K = plan["K"]
        GD = G + 1
        with (
            tc.tile_pool(name="const", bufs=1) as cpool,
            tc.tile_pool(name="slab", bufs=4) as slabpool,
            tc.tile_pool(name="oslab", bufs=8) as opool,
            tc.tile_pool(name="sixp", bufs=4) as sixpool,
            tc.tile_pool(name="psum", bufs=8, space="PSUM") as pspool,
        ):
            w_sb = cpool.tile([64, K * Cout], BF16, tag="w")
            nc.sync.dma_start(out=w_sb[:, :], in_=wt[:, :])
            for (s0seg, s1seg) in supers:
                a = seg_bounds[s0seg]
                b = seg_bounds[s1seg]
                ns_sup = b - a
                g = slabpool.tile([64, SUPER_SEGS * plan["seg_slots"]],
                                  BF16, tag="g")
                nc.sync.dma_start(out=g[:, :ns_sup], in_=slabt[:, a:b])
                si_t = sixpool.tile(
                    [128, SUPER_SEGS * plan["seg_slots"] // 16], I16,
                    tag="si")
                nc.sync.dma_start(
                    out=si_t[:, :ns_sup // 16],
                    in_=sidx[:, a // 16:b // 16],
                )
                for seg in range(s0seg, s1seg):
                    sa = seg_bounds[seg]
                    sb = seg_bounds[seg + 1]
                    ns = sb - sa
                    ntile = ns // 128
                    oslab = opool.tile([128, SEG_TILES, Cout], BF16,
                                       tag="oslab")
                    # batch 6 matmul outputs per PSUM bank; one wide
                    # drain copy each (amortizes the per-op overhead)
                    for g0 in range(0, ntile, 6):
                        g1 = min(g0 + 6, ntile)
                        ps = pspool.tile([128, 6, Cout], F32, tag="ps")
                        for t in range(g0, g1):
                            col = (sa - a) + t * 128
                            k = int(tile_k[sa // 128 + t])
                            nc.tensor.matmul(
                                out=ps[:, t - g0, :],
                                lhsT=g[:, col:col + 128],
                                rhs=w_sb[:, k * Cout:(k + 1) * Cout],
                                start=True, stop=True,
                            )
                        if g0 % 12 == 0:  # split drains across DVE/Act
                            nc.vector.tensor_copy(
                                out=oslab[:, g0:g1, :],
                                in_=ps[:, :g1 - g0, :])
                        else:
                            nc.scalar.activation(
                                out=oslab[:, g0:g1, :],
                                in_=ps[:, :g1 - g0, :],
                                func=mybir.ActivationFunctionType.Copy)
                    acc_e, acc_o = acc_eo[seg % NBANKS]
                    nc.gpsimd.dma_scatter_add(
                        out_ap=acc_e[:, :],
                        in_ap=oslab[:, :ntile, :],
                        idxs_ap=si_t[:, (sa - a) // 16:(sb - a) // 16],
                        num_idxs=ns,
                        num_idxs_reg=ns,
                        elem_size=Cout,
                        sbuf_tokens_per_rank=128,
                        parity_reg=int(os.environ.get("DECONV_PARITY", "0")),
                        out_ap_other=acc_o[:, :],
                    )

        # ---- BN phase ----
        with (
            tc.tile_pool(name="bn", bufs=1) as bnpool,
            tc.tile_pool(name="bns", bufs=4) as bnspool,
            tc.tile_pool(name="bnp", bufs=2, space="PSUM") as bnps,
        ):
            ones = bnpool.tile([128, 1], BF16, tag="ones")
            nc.vector.memset(ones[:, :], 1.0)
            RS = int(os.environ.get("DECONV_RS", "7"))  # stats batch width
            sum_ps = bnps.tile([1, RS * 64], F32, tag="sum")
            sq_ps = bnps.tile([1, RS * 64], F32, tag="sq")
            # fold banks 1.. into bank 0 in place (per parity, skip dump g)
            fold_e, fold_o = acc_eo[0]
            for b in range(1, NBANKS):
                nc.vector.tensor_tensor(
                    out=fold_e[:, :G * 64], in0=fold_e[:, :G * 64],
                    in1=acc_eo[b][0][:, :G * 64], op=mybir.AluOpType.add)
                nc.vector.tensor_tensor(
                    out=fold_o[:, :G * 64], in0=fold_o[:, :G * 64],
                    in1=acc_eo[b][1][:, :G * 64], op=mybir.AluOpType.add)
            folds = (fold_e, fold_o)
            nchunk = 2 * (G // RS)
            with tc.tile_pool(name="bnsq", bufs=4) as sqpool:
                for ci in range(nchunk):
                    p, q0 = ci % 2, (ci // 2) * RS
                    src = folds[p][:, q0 * 64:(q0 + RS) * 64]
                    sqt = sqpool.tile([128, RS * 64], BF16, tag="sqt")
                    if ci % 4 == 0:
                        nc.vector.tensor_tensor(
                            out=sqt[:, :], in0=src, in1=src,
                            op=mybir.AluOpType.mult)
                    else:
                        nc.scalar.activation(
                            out=sqt[:, :], in_=src,
                            func=mybir.ActivationFunctionType.Square)
                    nc.tensor.matmul(
                        out=sum_ps[:, :], lhsT=ones[:, :], rhs=src,
                        start=(ci == 0), stop=(ci == nchunk - 1),
                    )
                    nc.tensor.matmul(
                        out=sq_ps[:, :], lhsT=ones[:, :], rhs=sqt[:, :],
                        start=(ci == 0), stop=(ci == nchunk - 1),
                    )
            # combine the RS sub-sums -> [1, 64]
            sw = bnspool.tile([1, RS * 64], F32, tag="sw")
            qw = bnspool.tile([1, RS * 64], F32, tag="qw")
            nc.vector.tensor_copy(out=sw[:, :], in_=sum_ps[:, :])
            nc.vector.tensor_copy(out=qw[:, :], in_=sq_ps[:, :])
            st0 = bnspool.tile([1, 64], F32, tag="st0")
            st1 = bnspool.tile([1, 64], F32, tag="st1")
            nc.vector.tensor_copy(out=st0[:, :], in_=sw[:, 0:64])
            nc.vector.tensor_copy(out=st1[:, :], in_=qw[:, 0:64])
            for r in range(1, RS):
                nc.vector.tensor_tensor(
                    out=st0[:, :], in0=st0[:, :],
                    in1=sw[:, r * 64:(r + 1) * 64], op=mybir.AluOpType.add)
                nc.vector.tensor_tensor(
                    out=st1[:, :], in0=st1[:, :],
                    in1=qw[:, r * 64:(r + 1) * 64], op=mybir.AluOpType.add)
            nc.sync.dma_start(out=cc_in[0:1, :], in_=st0[:, :])
            nc.sync.dma_start(out=cc_in[1:2, :], in_=st1[:, :])
            nc.gpsimd.collective_compute(
                "AllReduce",
                mybir.AluOpType.add,
                ins=[cc_in[:, :]],
                outs=[cc_out[:, :]],
                replica_groups=[list(range(n_cores))],
            )
            gs0 = bnspool.tile([1, 64], F32, tag="gs0")
            gs1 = bnspool.tile([1, 64], F32, tag="gs1")
            nc.sync.dma_start(out=gs0[:, :], in_=cc_out[0:1, :])
            nc.sync.dma_start(out=gs1[:, :], in_=cc_out[1:2, :])
            gam_t = bnspool.tile([1, 64], F32, tag="gam")
            bet_t = bnspool.tile([1, 64], F32, tag="bet")
            nc.sync.dma_start(out=gam_t[:, :], in_=gb[0:1, :])
            nc.sync.dma_start(out=bet_t[:, :], in_=gb[1:2, :])

            inv_n = 1.0 / float(n_out)
            mean_t = bnspool.tile([1, 64], F32, tag="mean")
            ex2_t = bnspool.tile([1, 64], F32, tag="ex2")
            var_t = bnspool.tile([1, 64], F32, tag="var")
            sd_t = bnspool.tile([1, 64], F32, tag="sd")
            rs_t = bnspool.tile([1, 64], F32, tag="rs")
            a_t = bnspool.tile([1, 64], F32, tag="a")
            b_t = bnspool.tile([1, 64], F32, tag="b")
            nc.vector.tensor_scalar_mul(mean_t[:, :], gs0[:, :], inv_n)
            nc.vector.tensor_scalar_mul(ex2_t[:, :], gs1[:, :], inv_n)
            nc.vector.tensor_tensor(
                out=var_t[:, :], in0=mean_t[:, :], in1=mean_t[:, :],
                op=mybir.AluOpType.mult)
            nc.vector.tensor_tensor(
                out=var_t[:, :], in0=ex2_t[:, :], in1=var_t[:, :],
                op=mybir.AluOpType.subtract)
            nc.vector.tensor_scalar_add(var_t[:, :], var_t[:, :], BN_EPS)
            nc.scalar.activation(
                out=sd_t[:, :], in_=var_t[:, :],
                func=mybir.ActivationFunctionType.Sqrt)
            nc.vector.reciprocal(out=rs_t[:, :], in_=sd_t[:, :])
            nc.vector.tensor_tensor(
                out=a_t[:, :], in0=gam_t[:, :], in1=rs_t[:, :],
                op=mybir.AluOpType.mult)
            nc.vector.tensor_tensor(
                out=b_t[:, :], in0=mean_t[:, :], in1=a_t[:, :],
                op=mybir.AluOpType.mult)
            nc.vector.tensor_tensor(
                out=b_t[:, :], in0=bet_t[:, :], in1=b_t[:, :],
                op=mybir.AluOpType.subtract)
            # broadcast [1,64] -> [128,64] via PE (ones[1,128]^T @ row)
            ones_row = bnspool.tile([1, 128], F32, tag="ones_row")
            nc.vector.memset(ones_row[:, :], 1.0)
            a_full = bnspool.tile([128, 64], BF16, tag="afull")
            b_full = bnspool.tile([128, 64], BF16, tag="bfull")
            ab_ps = bnps.tile([128, 64], F32, tag="abps")
            nc.tensor.matmul(
                out=ab_ps[:, :], lhsT=ones_row[:, :], rhs=a_t[:, :],
                start=True, stop=True)
            nc.vector.tensor_copy(out=a_full[:, :], in_=ab_ps[:, :])
            nc.tensor.matmul(
                out=ab_ps[:, :], lhsT=ones_row[:, :], rhs=b_t[:, :],
                start=True, stop=True)
            nc.vector.tensor_copy(out=b_full[:, :], in_=ab_ps[:, :])
            # tile a,b 6 groups wide so normalize runs 6 groups per op
            REP = 6
            a_rep = bnspool.tile([128, REP * 64], BF16, tag="arep")
            b_rep = bnspool.tile([128, REP * 64], BF16, tag="brep")
            for r in range(REP):
                nc.vector.tensor_copy(
                    out=a_rep[:, r * 64:(r + 1) * 64], in_=a_full[:, :])
                nc.vector.tensor_copy(
                    out=b_rep[:, r * 64:(r + 1) * 64], in_=b_full[:, :])
            # normalize in place (bf16), relu into staging, one y write
            # per parity; the host un-permutes rows (y_perm)
            with tc.tile_pool(name="bny", bufs=2) as ypool:
                for p in range(2):
                    stage = ypool.tile([128, G, 64], BF16, tag="stage")
                    for i, q0 in enumerate(range(0, G, REP)):
                        qq = min(REP, G - q0)
                        src = folds[p][:, q0 * 64:(q0 + qq) * 64]
                        nc.vector.tensor_tensor(
                            out=src, in0=src, in1=a_rep[:, :qq * 64],
                            op=mybir.AluOpType.mult)
                        nc.vector.tensor_tensor(
                            out=src, in0=src, in1=b_rep[:, :qq * 64],
                            op=mybir.AluOpType.add)
                        dst = stage[:, q0:q0 + qq, :]
                        if i % 2 == 0:  # relu split across DVE/Act
                            nc.vector.tensor_scalar_max(dst, src, 0.0)
                        else:
                            nc.scalar.activation(
                                out=dst, in_=src,
                                func=mybir.ActivationFunctionType.Relu)
                    nc.sync.dma_start(
                        out=y[p * G * 128:(p + 1) * G * 128, :],
                        in_=stage[:, :, :])


def _pack_sidx(svals):
    """[total_slots] int -> [128, total_slots//16] int16 wrapped/tiled."""
    cols = svals.reshape(-1, 16).T.astype(np.int16)  # [16, n/16]
    return np.tile(cols, (8, 1))


def _prepare(feats, W, gamma, beta, in_map, out_map, n_out, n_cores=8,
             *_ignored):
    """Host prep shared by kernel() and tests. Returns (nc, in_maps, plan)."""
    n_out = int(n_out)
    K, Cin, Cout = W.shape
    assert Cin == 64 and Cout == 64
    in_map = np.asarray(in_map, dtype=np.int64)
    out_map = np.asarray(out_map, dtype=np.int64)
    feats = np.asarray(feats, dtype=np.float32)
    W = np.asarray(W, dtype=np.float32)

    plan, per_core = _route(in_map, out_map, n_out, n_cores)

    featsT = np.ascontiguousarray(
        feats.T.astype(ml_dtypes.bfloat16))          # [64, N_in]
    featsT_pad = np.concatenate(
        [featsT, np.zeros((64, 1), dtype=ml_dtypes.bfloat16)], axis=1)

    wt = np.ascontiguousarray(
        W.transpose(1, 0, 2).reshape(64, K * 64).astype(ml_dtypes.bfloat16))

    gb = np.stack([np.asarray(gamma, np.float32),
                   np.asarray(beta, np.float32)])

    nc = _build(plan, n_out, n_cores)
    in_maps = []
    for c in range(n_cores):
        gvals, svals, _ = per_core[c]
        slabt = featsT_pad[:, gvals]                 # -1 -> zero column
        in_maps.append(dict(slabt=np.ascontiguousarray(slabt), wt=wt,
                            sidx=_pack_sidx(svals), gb=gb))
    return nc, in_maps, plan


def kernel(feats, W, gamma, beta, in_map, out_map, n_out):
    from concourse.bass_utils import run_bass_kernel_spmd

    n_cores = 8
    nc, in_maps, plan = _prepare(
        feats, W, gamma, beta, in_map, out_map, n_out, n_cores)
    res = run_bass_kernel_spmd(nc, in_maps, list(range(n_cores)))
    rows = plan["rows_per_core"]
    perm = plan["y_perm"]
    out = np.concatenate(
        [res.results[c]["y"][perm][:rows] for c in range(n_cores)], axis=0)
    return out.astype(np.float32)
